# revision 1
# baseline (speedup 1.0000x reference)
"""Distributed attention kernel for 8 TRN2 NeuronCores.

Problem: B=2, N=M=4096, C=512, H=8 heads, D=64.
  q/k/v = linear(query/key/value); attn = softmax(q k^T / sqrt(D)); out = proj(attn v) + bias

Sharding: 1 head per core (tensor parallel over heads); each core runs both
batches for its head. A single 8-core AllToAll at the end swaps head-shards
for (batch, token-slice) shards: dest core j<4 gets batch-0 token-slice j,
core j>=4 gets batch-1 slice j-4; each core runs the output projection for
exactly one slice. The host gathers out from core j into full[b, slice].

Layouts (per core, head h = core index):
  inputs staged chunk-major + bf16 on host: [8, 128, 4, 512]
    (chunk j, partition p, t, n) = x.T[t*128+p, j*512+n] so each 512-column
    chunk load is one contiguous-per-partition DMA (cheap descriptors).
  qT, kT: (D=64, N) = WxT.T @ xT            (PE, bf16; q/k pairs run as
                                             concurrent column-tiles when the
                                             input chunks are resident)
  v:      (M, D) natural + a ones column     (so P@V also yields softmax denom)
  sT:     (m-tile 128, n-chunk 1024) = k q^T (PSUM f32, two 512 matmuls)
  P:      exp(sT / sqrt(D))                  (one wide ACT op; no max-sub:
                                              scores ~ N(0,1), max ~6)
  oT:     (65, 512) += v_aug.T @ P-half      (row 64 = sum_m P = softmax denom)
  xT:     oT[0:64] * (1/denom)               (denom bcast via GPSIMD
                                              partition_broadcast, recip on DVE)
  A2A:    shard j = xT_b(j)[:, slice]        (bf16, 1MB/rank, one collective)
  proj:   out(c', n) = WpT.T @ xfull + bp    (bias per-partition on ACT)
"""

import numpy as np
import ml_dtypes

import concourse.bass as bass
import concourse.mybir as mybir
import concourse.tile as tile
from concourse import bacc
from concourse.bass import ds, ts
from concourse.bass_utils import run_bass_kernel_spmd

BF16 = mybir.dt.bfloat16
F32 = mybir.dt.float32

B, N, M, C, H, D = 2, 4096, 4096, 512, 8, 64
SCALE = D ** -0.5
NCORES = 8
NSLICE = N // 4          # 1024 tokens per core after A2A
WCH = 1024               # scores/exp chunk width
NCH = 8                  # 512-column input chunks per tensor
EXP_FUNC = mybir.ActivationFunctionType.Exp
ID_FUNC = mybir.ActivationFunctionType.Identity
I16 = mybir.dt.int16
# DVE takes the top XDVE columns of each 1024-wide exp via a bf16 Schraudolph:
# bf16bits(exp(s/8)) ~= round(s * (0.125*128*log2 e) + (127*128 + 0.75)).
# The sawtooth error (~3% on P) mostly cancels in the softmax ratio; the
# uniform-scale part cancels exactly.
XDVE = 448
XACT = WCH - XDVE
SCHRA_A = float(0.125 * 128 * np.log2(np.e))
SCHRA_B = float(127 * 128 + 0.75)


def build_nc():
    nc = bacc.Bacc(
        "TRN2", target_bir_lowering=False, debug=False, num_devices=NCORES
    )

    # Per-core DRAM parameters. Chunk-major activations are identical on every
    # core; weight slices are per-head.
    qryT = [nc.declare_dram_parameter(f"queryT{b}", [NCH, 128, 4, 512], BF16, isOutput=False) for b in range(B)]
    keyT = [nc.declare_dram_parameter(f"keyT{b}", [NCH, 128, 4, 512], BF16, isOutput=False) for b in range(B)]
    valT = [nc.declare_dram_parameter(f"valueT{b}", [NCH, 128, 4, 512], BF16, isOutput=False) for b in range(B)]
    wqT = nc.declare_dram_parameter("wqT", [128, 4, D], BF16, isOutput=False)
    wkT = nc.declare_dram_parameter("wkT", [128, 4, D], BF16, isOutput=False)
    wvT = nc.declare_dram_parameter("wvT", [128, 4, D], BF16, isOutput=False)
    wpT = nc.declare_dram_parameter("wpT", [128, 4, C], BF16, isOutput=False)
    bp = nc.declare_dram_parameter("bp", [128, 4], F32, isOutput=False)
    vones = nc.declare_dram_parameter("vones", [128, 32], BF16, isOutput=False)
    out_a = nc.declare_dram_parameter("out_a", [C, NSLICE], F32, isOutput=True)
    out_b = nc.declare_dram_parameter("out_b", [C, NSLICE], F32, isOutput=True)

    with tile.TileContext(nc) as tc:
        with (
            tc.tile_pool(name="consts", bufs=1) as consts,
            tc.tile_pool(name="inputs", bufs=1) as inputs,
            tc.tile_pool(name="qk", bufs=2) as qkpool,
            tc.tile_pool(name="vp", bufs=2) as vpool,
            tc.tile_pool(name="ppool", bufs=3) as ppool,
            tc.tile_pool(name="small", bufs=2) as small,
            tc.tile_pool(name="xt", bufs=1) as xtpool,
            tc.tile_pool(name="psS", bufs=2, space="PSUM") as psS,   # scores, 2 banks/slot
            tc.tile_pool(name="psO", bufs=1, space="PSUM") as psO,   # attention accumulators
            tc.tile_pool(name="psB", bufs=1, space="PSUM") as psB,   # qkv/proj matmuls
            tc.tile_pool(name="dram", bufs=1, space="DRAM") as dram,
        ):
            # ---- constants / weights (host pre-tiled: plain contiguous loads)
            wq_sb = consts.tile([128, 4, D], BF16, name="wq_sb")
            wk_sb = consts.tile([128, 4, D], BF16, name="wk_sb")
            wv_sb = consts.tile([128, 4, D], BF16, name="wv_sb")
            wp_sb = consts.tile([128, 4, C], BF16, name="wp_sb")
            bp_sb = consts.tile([128, 4], F32, name="bp_sb")
            nc.sync.dma_start(out=wq_sb, in_=wqT[:])
            nc.sync.dma_start(out=wk_sb, in_=wkT[:])
            nc.sync.dma_start(out=wv_sb, in_=wvT[:])
            nc.sync.dma_start(out=wp_sb, in_=wpT[:])
            nc.sync.dma_start(out=bp_sb, in_=bp[:])

            # Dest-split A2As: shard j<4 carries batch-0 token-slice j, shard
            # j>=4 carries batch-1 token-slice j-4 — a 1:1 (batch, slice) to
            # core assignment, so each core runs ONE slice's projection.
            # A2A #1 omits the chunk-3-dependent shard (dest 7) and fires
            # after batch-1 chunk 2: its setup/barrier/transfer and the
            # projection for cores 0-6 all hide under attn1's last chunk.
            # A2A #2 (only shard 7 meaningful) is the sole exposed tail.
            a2a_in = [dram.tile([NCORES, D, NSLICE], BF16, name=f"a2a_in{x}") for x in range(2)]
            a2a_out = [dram.tile([NCORES, D, NSLICE], BF16, name=f"a2a_out{x}") for x in range(2)]

            def once(fn):
                done = [False]
                def wrapper():
                    if not done[0]:
                        done[0] = True
                        fn()
                return wrapper

            def qkv_units(b, paired, defer_loads=False):
                """Allocate tiles; return (loads, unit closures).

                Each unit closure emits one PSUM-sized piece of the q/k/v
                projections. The caller weaves them into the attention loop so
                the in-order PE stream never sits on a long block of
                projection matmuls while ACT is idle.

                paired: emit q+k of the same chunk as two concurrent PE
                column-tiles (outputs to partition halves of one PSUM bank) —
                ~2x faster, but couples their input-chunk deadlines, so it is
                only used when the input data is already resident (batch 1)
                or guaranteed-early (first two chunks of batch 0).
                """
                qry_sb = inputs.tile([128, NCH, 4, 512], BF16, name="qry_sb")
                key_sb = inputs.tile([128, NCH, 4, 512], BF16, name="key_sb")
                val_sb = inputs.tile([128, NCH, 4, 512], BF16, name="val_sb")
                qT_sb = qkpool.tile([D, N], BF16, name="qT_sb", tag="qT_sb")
                kT_sb = qkpool.tile([D, M], BF16, name="kT_sb", tag="kT_sb")
                v_sb = vpool.tile([128, 32, D + 1], BF16, name="v_sb")

                @once
                def loads():
                    # chunk loads are contiguous per partition (1 descriptor
                    # row): issue cost ~100ns each instead of 0.7-6us.
                    # Order: the pre-loop units (kq0, kq1, v0-3) first, then
                    # key/val interleaved (their use-deadlines advance with
                    # the m-tile loop), q chunks 2+ last (needed a full
                    # scores-chunk later).
                    with nc.named_scope(f"qkv{b}"):
                        def chunk(dst, src, j):
                            nc.sync.dma_start(out=dst[:, j], in_=src[j])
                        chunk(key_sb, keyT[b], 0)
                        chunk(qry_sb, qryT[b], 0)
                        chunk(key_sb, keyT[b], 1)
                        chunk(qry_sb, qryT[b], 1)
                        chunk(val_sb, valT[b], 0)
                        chunk(val_sb, valT[b], 1)
                        nc.sync.dma_start(out=v_sb[:, :, D], in_=vones[:])
                        for j in range(2, NCH):
                            chunk(key_sb, keyT[b], j)
                            chunk(val_sb, valT[b], j)
                        for j in range(2, NCH):
                            chunk(qry_sb, qryT[b], j)
                if not defer_loads:
                    loads()

                def pair_unit(s1, s2):
                    # two 64-row projection pieces as concurrent column-tiles:
                    # s1 -> output partitions 0:64 (PE col groups 0-1),
                    # s2 -> 64:128 (groups 2-3); disjoint sub-arrays run
                    # simultaneously, so the pair costs ~one unit.
                    (d1, w1, s1_, i1), (d2, w2, s2_, i2) = s1, s2
                    def emit():
                        ps = psB.tile([128, 512], F32, name="pskq", tag="psB")
                        for t in range(4):
                            nc.tensor.matmul(
                                ps[0:D, :], w1[:, t, :], s1_[:, i1, t, :],
                                start=(t == 0), stop=(t == 3),
                            )
                            nc.tensor.matmul(
                                ps[D:128, :], w2[:, t, :], s2_[:, i2, t, :],
                                start=(t == 0), stop=(t == 3),
                            )
                        with nc.allow_low_precision(reason="bf16 scores"):
                            nc.vector.tensor_copy(d1[:, ds(i1 * 512, 512)], ps[0:D, :])
                            nc.vector.tensor_copy(d2[:, ds(i2 * 512, 512)], ps[D:128, :])
                    return emit

                def K(i):
                    return (kT_sb, wk_sb, key_sb, i)

                def Q(i):
                    return (qT_sb, wq_sb, qry_sb, i)

                def v_unit(mt):
                    def emit():
                        psv = psB.tile([128, D], F32, name="psv", tag="psB")
                        for t in range(4):
                            nc.tensor.matmul(
                                psv, val_sb[:, mt // 4, t, ds((mt % 4) * 128, 128)],
                                wv_sb[:, t, :],
                                start=(t == 0), stop=(t == 3),
                            )
                        nc.vector.tensor_copy(v_sb[:, mt, 0:D], psv)
                    return emit

                if paired:
                    # batch-1 fillers: all inputs resident, pair freely
                    pre = [once(pair_unit(K(i), Q(i))) for i in range(NCH)]
                    sched = []
                else:
                    # batch 0 (cold start): chunks 0/1 arrive first and pair
                    # k+q; later chunks pair k+k (key data lands early) and
                    # q+q (both deadlines a full scores-chunk away), so every
                    # unit's inputs beat its weave slot.
                    pre = [once(pair_unit(K(0), Q(0))), once(pair_unit(K(1), Q(1)))]
                    # weave each pair at the latest slot that still beats its
                    # consumer (k chunks i feed scores m-tiles 4i): later
                    # slots let ready scores/PV work run ahead of the input
                    # DMA instead of stalling the PE queue on it.
                    sched = [
                        (4, once(pair_unit(K(2), K(3)))),
                        (10, once(pair_unit(K(4), K(5)))),
                        (16, once(pair_unit(K(6), K(7)))),
                        (20, once(pair_unit(Q(2), Q(3)))),
                        (24, once(pair_unit(Q(4), Q(5)))),
                        (28, once(pair_unit(Q(6), Q(7)))),
                    ]
                v_units = [once(v_unit(mt)) for mt in range(32)]
                return loads, qT_sb, kT_sb, v_sb, pre, sched, v_units

            def proj_units(x, out_ext):
                """A2A #x + xf-load closure, plus this core's projection."""
                xf = [
                    xtpool.tile([128, NSLICE], BF16, name=f"xf{x}{t}", tag=f"xf{t}")
                    for t in range(4)
                ]

                @once
                def comm():
                    with nc.named_scope(f"proj{x}"):
                        nc.gpsimd.collective_compute(
                            "AllToAll",
                            mybir.AluOpType.bypass,
                            replica_groups=[list(range(NCORES))],
                            ins=[a2a_in[x].opt()],
                            outs=[a2a_out[x].opt()],
                        )
                        for t in range(4):
                            nc.sync.dma_start(
                                out=xf[t],
                                in_=a2a_out[x][2 * t : 2 * t + 2].rearrange("s d n -> (s d) n"),
                            )

                def p_unit(ct, hf):
                    def emit():
                        psp = psB.tile([128, 512], F32, name="psp", tag="psB")
                        for et in range(4):
                            nc.tensor.matmul(
                                psp, wp_sb[:, et, ts(ct, 128)], xf[et][:, ts(hf, 512)],
                                start=(et == 0), stop=(et == 3),
                            )
                        out_sb = small.tile([128, 512], F32, name="out_sb", bufs=4)
                        nc.scalar.activation(out_sb, psp, ID_FUNC, bias=bp_sb[:, ct : ct + 1])
                        nc.sync.dma_start(out=out_ext[ts(ct, 128), ts(hf, 512)], in_=out_sb)
                    return emit

                return comm, [once(p_unit(ct, hf)) for ct in range(4) for hf in range(2)]

            def do_attn(b, qkv, own_weave, fillers, fill_from_wci, carry=(),
                        defer_last=False, tail_weave=()):
                """Attention for batch b, software-pipelined.

                own_weave: weave this batch's own k/q/v units in by deadline
                (first-batch cold start). fillers: unconstrained unit closures
                (next batch's qkv) drained one per m-tile starting at chunk
                fill_from_wci. tail_weave: (mt, closure) pairs placed in the
                LAST chunk's loop (the hidden A2A + its projection).
                """
                loads, qT_sb, kT_sb, v_sb, pre, sched, v_units = qkv
                if not own_weave:
                    for u in pre + [u for _, u in sched] + v_units:
                        u()  # one-shot: no-op for units already woven elsewhere
                else:
                    for u in pre:
                        u()
                    for mt in range(4):
                        v_units[mt]()

                xT_sb = xtpool.tile([D, N], BF16, name="xT_sb", tag="xT_sb", bufs=1)
                with nc.named_scope(f"attn{b}"):
                    def scores(wci, mt):
                        pss = psS.tile([128, WCH], F32, name="pss", tag="pss")
                        for h in range(2):
                            nc.tensor.matmul(
                                pss[:, ts(h, 512)],
                                kT_sb[:, ts(mt, 128)],
                                qT_sb[:, ds(wci * WCH + h * 512, 512)],
                                start=True, stop=True,
                            )
                        return pss

                    def make_norm(o_sb, r_sb, wci):
                        # deferred normalize: xT = o_sb * (1/r_sb); woven into
                        # the NEXT chunk's loop. The denominator row is
                        # replicated across D partitions on the (idle) GPSIMD
                        # engine so the PE never touches the normalize.
                        rbs = []

                        def stage_a():
                            for h in range(2):
                                rb = small.tile([D, 512], F32, name="rb", bufs=4)
                                nc.gpsimd.partition_broadcast(rb, r_sb[h], channels=D)
                                rbs.append(rb)

                        def stage_b():
                            for h in range(2):
                                rbi = small.tile([D, 512], F32, name="rbi", bufs=4)
                                nc.vector.reciprocal_approx_fast(rbi, rbs[h])
                                nc.vector.tensor_mul(
                                    xT_sb[:, ds(wci * WCH + h * 512, 512)],
                                    o_sb[h], rbi,
                                )
                            if b == 1:
                                # scatter this chunk's shard right away: the
                                # hidden A2A #1 takes slices 0-2, the tail
                                # A2A #2 takes only the last chunk's slice.
                                nc.sync.dma_start(
                                    out=a2a_in[0 if wci < 3 else 1][4 + wci],
                                    in_=xT_sb[:, ds(wci * WCH, WCH)],
                                )
                        return [stage_a, stage_b]

                    # software-pipelined: scores(mt+1) is emitted before PV(mt)
                    # so the in-order PE stream never stalls behind the exp.
                    pss_cur = scores(0, 0)
                    pending_norm = None
                    for wci in range(N // WCH):
                        pso = [
                            psO.tile([D + 1, 512], F32, name=f"pso{h}", tag=f"pso{h}",
                                     bufs=2 if h == 0 else 1)
                            for h in range(2)
                        ]
                        pv0 = None  # deferred first PV of the chunk
                        for mt in range(32):
                            if wci == N // WCH - 1:
                                for wmt, u in tail_weave:
                                    if wmt == mt:
                                        u()
                            if pending_norm is not None and mt in (2, 4):
                                pending_norm[(mt - 2) // 2]()
                            if carry and wci == 0 and mt in (6, 8, 10, 12):
                                ci = (mt - 6) // 2
                                if ci < len(carry):
                                    carry[ci]()
                            if own_weave and wci == 0:
                                # cold start: feed k/q/v units just ahead of
                                # use; all own units are done by end of wci 0
                                # so fillers (from wci 1) can't pool-deadlock.
                                for wmt, u in sched:
                                    if wmt == mt:
                                        u()
                                if mt + 4 < 32:
                                    v_units[mt + 4]()
                            if wci >= fill_from_wci and fillers:
                                fillers.pop(0)()
                            p_sb = ppool.tile([128, WCH], BF16, name="p_sb")
                            nc.scalar.activation(
                                p_sb[:, 0:XACT], pss_cur[:, 0:XACT], EXP_FUNC, scale=SCALE
                            )
                            if XDVE:
                                with nc.allow_low_precision(reason="schraudolph exp"):
                                    nc.vector.tensor_scalar(
                                        p_sb.bitcast(I16)[:, XACT:WCH],
                                        pss_cur[:, XACT:WCH],
                                        SCHRA_A, SCHRA_B,
                                        op0=mybir.AluOpType.mult,
                                        op1=mybir.AluOpType.add,
                                    )
                            if mt < 31:
                                pss_cur = scores(wci, mt + 1)
                            elif wci < N // WCH - 1:
                                pss_cur = scores(wci + 1, 0)
                            if mt == 0:
                                # defer PV(0) one iteration: the PE keeps two
                                # scores queued while the previous chunk's pso
                                # slots drain through their SBUF copies.
                                pv0 = p_sb
                                continue
                            if mt == 1:
                                for h in range(2):
                                    nc.tensor.matmul(
                                        pso[h], v_sb[:, 0, :], pv0[:, ts(h, 512)],
                                        start=True, stop=False,
                                    )
                            for h in range(2):
                                nc.tensor.matmul(
                                    pso[h], v_sb[:, mt, :], p_sb[:, ts(h, 512)],
                                    start=False, stop=(mt == 31),
                                )
                        # free the pso slots promptly via PSUM->SBUF copies;
                        # the rest of the normalize is deferred into the next
                        # chunk (or emitted now for the last chunk). The denom
                        # row is copied to a partition-0 tile because the
                        # custom-DVE reciprocal can't take partition offsets.
                        o_sb, r_sb = [], []
                        for h in range(2):
                            o = small.tile([D, 512], F32, name=f"o_sb{h}", tag=f"o_sb{h}")
                            nc.vector.tensor_copy(o, pso[h][0:D, :])
                            r = small.tile([1, 512], F32, name=f"r_sb{h}", tag=f"r_sb{h}")
                            nc.vector.tensor_copy(r, pso[h][D : D + 1, :])
                            o_sb.append(o)
                            r_sb.append(r)
                        pending_norm = make_norm(o_sb, r_sb, wci)
                        if wci == N // WCH - 1 and not defer_last:
                            pending_norm[0]()
                            pending_norm[1]()
                            pending_norm = None

                @once
                def scatter():
                    if b == 1:
                        return  # batch-1 shards scatter per-chunk in stage_b
                    for j in range(4):
                        nc.sync.dma_start(
                            out=a2a_in[0][j],
                            in_=xT_sb[:, ds(j * NSLICE, NSLICE)],
                        )

                if defer_last:
                    # hand the last chunk's normalize + this batch's scatter
                    # to the caller to weave into the next batch's attention,
                    # keeping the batch transition stall-free.
                    return [once(pending_norm[0]), once(pending_norm[1]), scatter]
                scatter()
                return []

            # Batch 0 weaves its own qkv units in by deadline (cold start);
            # batch 1's loads + qkv units fill batch 0's ACT-bound attention.
            # The combined A2A + this core's single-slice projection are the
            # only tail after batch 1's attention.
            qkv0 = qkv_units(0, paired=False)
            qkv1 = qkv_units(1, paired=True, defer_loads=True)
            fill0 = [qkv1[0]] + qkv1[4] + [u for _, u in qkv1[5]] + qkv1[6]
            carry0 = do_attn(0, qkv0, own_weave=True, fillers=fill0,
                             fill_from_wci=1, defer_last=True)
            for u in fill0:
                u()
            # A2A #1's trigger fires inside attn1's last chunk (its inputs
            # complete with chunk 2's deferred normalize at mt 4), so its
            # setup/barrier/transfer run on the CC engine under the last
            # chunk's compute without ever blocking the PE. The projection
            # off it (valid on cores 0-6) runs right after attn1, filling
            # A2A #2's barrier wait with real PE work — A2A #1's data always
            # lands before A2A #2's (same straggler, one chunk earlier).
            # Core 7 (whose slice IS the last chunk) projects off A2A #2.
            comm1, pu1 = proj_units(0, out_a)
            do_attn(1, qkv1, own_weave=False, fillers=[],
                    fill_from_wci=2, carry=carry0, tail_weave=[(6, comm1)])
            for u in carry0:
                u()
            comm2, pu2 = proj_units(1, out_b)
            comm2()
            for u in pu1:
                u()
            for u in pu2:
                u()

    nc.finalize()
    return nc


_NC_CACHE = {}


def _get_nc():
    if "nc" not in _NC_CACHE:
        _NC_CACHE["nc"] = build_nc()
    return _NC_CACHE["nc"]


def _chunk_major(xT):
    # [C, N] -> [8, 128, 4, 512]: (j, p, t, n) = xT[t*128+p, j*512+n]
    return np.ascontiguousarray(
        xT.reshape(4, 128, NCH, 512).transpose(2, 1, 0, 3)
    )


def _w_tiled(wT, d):
    # [C, d] -> [128, 4, d]: (p, t, :) = wT[t*128+p, :]
    return np.ascontiguousarray(wT.reshape(4, 128, d).transpose(1, 0, 2))


def _make_in_maps(query, key, value, Wq, Wk, Wv, Wp, bp):
    bf = ml_dtypes.bfloat16
    shared = {}
    for b in range(B):
        shared[f"queryT{b}"] = _chunk_major(query[b].T.astype(bf))
        shared[f"keyT{b}"] = _chunk_major(key[b].T.astype(bf))
        shared[f"valueT{b}"] = _chunk_major(value[b].T.astype(bf))
    shared["wpT"] = _w_tiled(Wp.T.astype(bf), C)
    shared["bp"] = np.ascontiguousarray(
        bp.astype(np.float32).reshape(4, 128).T
    )
    shared["vones"] = np.ones((128, 32), bf)

    in_maps = []
    for j in range(NCORES):
        m = dict(shared)
        m["wqT"] = _w_tiled(Wq[j * D : (j + 1) * D, :].T.astype(bf), D)
        m["wkT"] = _w_tiled(Wk[j * D : (j + 1) * D, :].T.astype(bf), D)
        m["wvT"] = _w_tiled(Wv[j * D : (j + 1) * D, :].T.astype(bf), D)
        in_maps.append(m)
    return in_maps


def run(inputs, trace=False):
    inputs = {k: np.asarray(v) for k, v in inputs.items()}
    nc = _get_nc()
    in_maps = _make_in_maps(**inputs)
    res = run_bass_kernel_spmd(nc, in_maps, core_ids=list(range(NCORES)), trace=trace)
    full = np.empty((B, N, C), np.float32)
    for s in range(4):
        full[0, s * NSLICE : (s + 1) * NSLICE, :] = res.results[s]["out_a"].T
    for s in range(3):
        full[1, s * NSLICE : (s + 1) * NSLICE, :] = res.results[4 + s]["out_a"].T
    full[1, 3 * NSLICE :, :] = res.results[7]["out_b"].T
    return full, res


def kernel(**inputs):
    return run(inputs, trace=False)[0]



# revision 4
# speedup vs baseline: 1.2236x; 1.2236x over previous
"""Distributed attention kernel for 8 TRN2 NeuronCores.

Problem: B=2, N=M=4096, C=512, H=8 heads, D=64.
  q/k/v = linear(query/key/value); attn = softmax(q k^T / sqrt(D)); out = proj(attn v) + bias

Sharding: 1 head per core (tensor parallel over heads); each core runs both
batches for its head. A single 8-core AllToAll at the end swaps head-shards
for (batch, token-slice) shards: dest core j<4 gets batch-0 token-slice j,
core j>=4 gets batch-1 slice j-4; each core runs the output projection for
exactly one slice. The host gathers out from core j into full[b, slice].

Layouts (per core, head h = core index):
  inputs staged chunk-major + bf16 on host: [8, 128, 4, 512]
    (chunk j, partition p, t, n) = x.T[t*128+p, j*512+n] so each 512-column
    chunk load is one contiguous-per-partition DMA (cheap descriptors).
  qT, kT: (D=64, N) = WxT.T @ xT            (PE, bf16; q/k pairs run as
                                             concurrent column-tiles when the
                                             input chunks are resident)
  v:      (M, D) natural + a ones column     (so P@V also yields softmax denom)
  sT:     (m-tile 128, n-chunk 1024) = k q^T (PSUM f32, two 512 matmuls)
  P:      exp(sT / sqrt(D))                  (one wide ACT op; no max-sub:
                                              scores ~ N(0,1), max ~6)
  oT:     (65, 512) += v_aug.T @ P-half      (row 64 = sum_m P = softmax denom)
  xT:     oT[0:64] * (1/denom)               (denom bcast via GPSIMD
                                              partition_broadcast, recip on DVE)
  A2A:    shard j = xT_b(j)[:, slice]        (bf16, 1MB/rank, one collective)
  proj:   out(c', n) = WpT.T @ xfull + bp    (bias per-partition on ACT)
"""

import numpy as np
import ml_dtypes

import concourse.bass as bass
import concourse.mybir as mybir
import concourse.tile as tile
from concourse import bacc
from concourse.bass import ds, ts
from concourse.bass_utils import run_bass_kernel_spmd

BF16 = mybir.dt.bfloat16
F32 = mybir.dt.float32

B, N, M, C, H, D = 2, 4096, 4096, 512, 8, 64
SCALE = D ** -0.5
NCORES = 8
NSLICE = N // 4          # 1024 tokens per core after A2A
WCH = 1024               # scores/exp chunk width
NCH = 8                  # 512-column input chunks per tensor
EXP_FUNC = mybir.ActivationFunctionType.Exp
ID_FUNC = mybir.ActivationFunctionType.Identity
I16 = mybir.dt.int16
# DVE takes the top XDVE columns of each 1024-wide exp via a bf16 Schraudolph:
# bf16bits(exp(s/8)) ~= round(s * (0.125*128*log2 e) + (127*128 + 0.75)).
# The sawtooth error (~3% on P) mostly cancels in the softmax ratio; the
# uniform-scale part cancels exactly.
XDVE = 448
XACT = WCH - XDVE
SCHRA_A = float(0.125 * 128 * np.log2(np.e))
SCHRA_B = float(127 * 128 + 0.75)


def build_nc():
    nc = bacc.Bacc(
        "TRN2", target_bir_lowering=False, debug=False, num_devices=NCORES
    )

    # Per-core DRAM parameters. Chunk-major activations are identical on every
    # core; weight slices are per-head.
    qryT = [nc.declare_dram_parameter(f"queryT{b}", [NCH, 128, 4, 512], BF16, isOutput=False) for b in range(B)]
    keyT = [nc.declare_dram_parameter(f"keyT{b}", [NCH, 128, 4, 512], BF16, isOutput=False) for b in range(B)]
    valT = [nc.declare_dram_parameter(f"valueT{b}", [NCH, 128, 4, 512], BF16, isOutput=False) for b in range(B)]
    wqT = nc.declare_dram_parameter("wqT", [128, 4, D], BF16, isOutput=False)
    wkT = nc.declare_dram_parameter("wkT", [128, 4, D], BF16, isOutput=False)
    wvT = nc.declare_dram_parameter("wvT", [128, 4, D], BF16, isOutput=False)
    wpT = nc.declare_dram_parameter("wpT", [128, 4, C], BF16, isOutput=False)
    bp = nc.declare_dram_parameter("bp", [128, 4], F32, isOutput=False)
    vones = nc.declare_dram_parameter("vones", [128, 32], BF16, isOutput=False)
    out_a = nc.declare_dram_parameter("out_a", [C, NSLICE], F32, isOutput=True)
    out_b = nc.declare_dram_parameter("out_b", [C, NSLICE], F32, isOutput=True)

    with tile.TileContext(nc) as tc:
        with (
            tc.tile_pool(name="consts", bufs=1) as consts,
            tc.tile_pool(name="inputs", bufs=1) as inputs,
            tc.tile_pool(name="qk", bufs=2) as qkpool,
            tc.tile_pool(name="vp", bufs=2) as vpool,
            tc.tile_pool(name="ppool", bufs=3) as ppool,
            tc.tile_pool(name="small", bufs=2) as small,
            tc.tile_pool(name="xt", bufs=1) as xtpool,
            tc.tile_pool(name="psS", bufs=2, space="PSUM") as psS,   # scores, 2 banks/slot
            tc.tile_pool(name="psO", bufs=1, space="PSUM") as psO,   # attention accumulators
            tc.tile_pool(name="psB", bufs=1, space="PSUM") as psB,   # qkv/proj matmuls
            tc.tile_pool(name="dram", bufs=1, space="DRAM") as dram,
        ):
            # ---- constants / weights (host pre-tiled: plain contiguous loads)
            wq_sb = consts.tile([128, 4, D], BF16, name="wq_sb")
            wk_sb = consts.tile([128, 4, D], BF16, name="wk_sb")
            wv_sb = consts.tile([128, 4, D], BF16, name="wv_sb")
            wp_sb = consts.tile([128, 4, C], BF16, name="wp_sb")
            bp_sb = consts.tile([128, 4], F32, name="bp_sb")
            nc.sync.dma_start(out=wq_sb, in_=wqT[:])
            nc.sync.dma_start(out=wk_sb, in_=wkT[:])
            nc.sync.dma_start(out=wv_sb, in_=wvT[:])
            nc.sync.dma_start(out=wp_sb, in_=wpT[:])
            nc.sync.dma_start(out=bp_sb, in_=bp[:])

            # Dest-split A2As: shard j<4 carries batch-0 token-slice j, shard
            # j>=4 carries batch-1 token-slice j-4 — a 1:1 (batch, slice) to
            # core assignment, so each core runs ONE slice's projection.
            # A2A #1 omits the chunk-3-dependent shard (dest 7) and fires
            # after batch-1 chunk 2: its setup/barrier/transfer and the
            # projection for cores 0-6 all hide under attn1's last chunk.
            # A2A #2 (only shard 7 meaningful) is the sole exposed tail.
            a2a_in = [dram.tile([NCORES, D, NSLICE], BF16, name=f"a2a_in{x}") for x in range(2)]
            a2a_out = [dram.tile([NCORES, D, NSLICE], BF16, name=f"a2a_out{x}") for x in range(2)]

            def once(fn):
                done = [False]
                def wrapper():
                    if not done[0]:
                        done[0] = True
                        fn()
                return wrapper

            def qkv_units(b, paired, defer_loads=False):
                """Allocate tiles; return (loads, unit closures).

                Each unit closure emits one PSUM-sized piece of the q/k/v
                projections. The caller weaves them into the attention loop so
                the in-order PE stream never sits on a long block of
                projection matmuls while ACT is idle.

                paired: emit q+k of the same chunk as two concurrent PE
                column-tiles (outputs to partition halves of one PSUM bank) —
                ~2x faster, but couples their input-chunk deadlines, so it is
                only used when the input data is already resident (batch 1)
                or guaranteed-early (first two chunks of batch 0).
                """
                qry_sb = inputs.tile([128, NCH, 4, 512], BF16, name="qry_sb")
                key_sb = inputs.tile([128, NCH, 4, 512], BF16, name="key_sb")
                val_sb = inputs.tile([128, NCH, 4, 512], BF16, name="val_sb")
                # qT/kT live duplicated in BOTH partition halves (rows 0:64 and
                # 64:128) so the two 512-wide score halves of each m-tile can
                # run as concurrent PE ROW-tiles (tile_position (0,0)/(64,0)):
                # K=64 only fills half the array, so the pair costs ~one unit.
                qT_sb = qkpool.tile([128, N], BF16, name="qT_sb", tag="qT_sb")
                kT_sb = qkpool.tile([128, M], BF16, name="kT_sb", tag="kT_sb")
                v_sb = vpool.tile([128, 32, D + 1], BF16, name="v_sb")

                @once
                def loads():
                    # chunk loads are contiguous per partition (1 descriptor
                    # row): issue cost ~100ns each instead of 0.7-6us.
                    # Order: the pre-loop units (kq0, kq1, v0-3) first, then
                    # key/val interleaved (their use-deadlines advance with
                    # the m-tile loop), q chunks 2+ last (needed a full
                    # scores-chunk later).
                    with nc.named_scope(f"qkv{b}"):
                        def chunk(dst, src, j):
                            nc.sync.dma_start(out=dst[:, j], in_=src[j])
                        chunk(key_sb, keyT[b], 0)
                        chunk(qry_sb, qryT[b], 0)
                        chunk(key_sb, keyT[b], 1)
                        chunk(qry_sb, qryT[b], 1)
                        chunk(val_sb, valT[b], 0)
                        chunk(val_sb, valT[b], 1)
                        nc.sync.dma_start(out=v_sb[:, :, D], in_=vones[:])
                        for j in range(2, NCH):
                            chunk(key_sb, keyT[b], j)
                            chunk(val_sb, valT[b], j)
                        for j in range(2, NCH):
                            chunk(qry_sb, qryT[b], j)
                if not defer_loads:
                    loads()

                def pair_unit(s1, s2):
                    # two 64-row projection pieces as concurrent column-tiles:
                    # s1 -> output partitions 0:64 (PE col groups 0-1),
                    # s2 -> 64:128 (groups 2-3); disjoint sub-arrays run
                    # simultaneously, so the pair costs ~one unit.
                    (d1, w1, s1_, i1), (d2, w2, s2_, i2) = s1, s2
                    def emit():
                        ps = psB.tile([128, 512], F32, name="pskq", tag="psB")
                        for t in range(4):
                            nc.tensor.matmul(
                                ps[0:D, :], w1[:, t, :], s1_[:, i1, t, :],
                                start=(t == 0), stop=(t == 3),
                            )
                            nc.tensor.matmul(
                                ps[D:128, :], w2[:, t, :], s2_[:, i2, t, :],
                                start=(t == 0), stop=(t == 3),
                            )
                        with nc.allow_low_precision(reason="bf16 scores"):
                            nc.vector.tensor_copy(d1[0:D, ds(i1 * 512, 512)], ps[0:D, :])
                            nc.vector.tensor_copy(d1[D:128, ds(i1 * 512, 512)], ps[0:D, :])
                            nc.vector.tensor_copy(d2[0:D, ds(i2 * 512, 512)], ps[D:128, :])
                            nc.vector.tensor_copy(d2[D:128, ds(i2 * 512, 512)], ps[D:128, :])
                    return emit

                def K(i):
                    return (kT_sb, wk_sb, key_sb, i)

                def Q(i):
                    return (qT_sb, wq_sb, qry_sb, i)

                def v_unit(mt):
                    def emit():
                        psv = psB.tile([128, D], F32, name="psv", tag="psB")
                        for t in range(4):
                            nc.tensor.matmul(
                                psv, val_sb[:, mt // 4, t, ds((mt % 4) * 128, 128)],
                                wv_sb[:, t, :],
                                start=(t == 0), stop=(t == 3),
                            )
                        nc.vector.tensor_copy(v_sb[:, mt, 0:D], psv)
                    return emit

                if paired:
                    # batch-1 fillers: all inputs resident, pair freely
                    pre = [once(pair_unit(K(i), Q(i))) for i in range(NCH)]
                    sched = []
                else:
                    # batch 0 (cold start): chunks 0/1 arrive first and pair
                    # k+q; later chunks pair k+k (key data lands early) and
                    # q+q (both deadlines a full scores-chunk away), so every
                    # unit's inputs beat its weave slot.
                    pre = [once(pair_unit(K(0), Q(0))), once(pair_unit(K(1), Q(1)))]
                    # weave each pair at the latest slot that still beats its
                    # consumer (k chunks i feed scores m-tiles 4i): later
                    # slots let ready scores/PV work run ahead of the input
                    # DMA instead of stalling the PE queue on it.
                    sched = [
                        (4, once(pair_unit(K(2), K(3)))),
                        (10, once(pair_unit(K(4), K(5)))),
                        (16, once(pair_unit(K(6), K(7)))),
                        (20, once(pair_unit(Q(2), Q(3)))),
                        (24, once(pair_unit(Q(4), Q(5)))),
                        (28, once(pair_unit(Q(6), Q(7)))),
                    ]
                v_units = [once(v_unit(mt)) for mt in range(32)]
                return loads, qT_sb, kT_sb, v_sb, pre, sched, v_units

            def proj_units(x, out_ext):
                """A2A #x + xf-load closure, plus this core's projection."""
                xf = [
                    xtpool.tile([128, NSLICE], BF16, name=f"xf{x}{t}", tag=f"xf{t}")
                    for t in range(4)
                ]

                @once
                def comm():
                    with nc.named_scope(f"proj{x}"):
                        nc.gpsimd.collective_compute(
                            "AllToAll",
                            mybir.AluOpType.bypass,
                            replica_groups=[list(range(NCORES))],
                            ins=[a2a_in[x].opt()],
                            outs=[a2a_out[x].opt()],
                        )
                        for t in range(4):
                            nc.sync.dma_start(
                                out=xf[t],
                                in_=a2a_out[x][2 * t : 2 * t + 2].rearrange("s d n -> (s d) n"),
                            )

                def p_unit(ct, hf):
                    def emit():
                        psp = psB.tile([128, 512], F32, name="psp", tag="psB")
                        for et in range(4):
                            nc.tensor.matmul(
                                psp, wp_sb[:, et, ts(ct, 128)], xf[et][:, ts(hf, 512)],
                                start=(et == 0), stop=(et == 3),
                            )
                        out_sb = small.tile([128, 512], F32, name="out_sb", bufs=4)
                        nc.scalar.activation(out_sb, psp, ID_FUNC, bias=bp_sb[:, ct : ct + 1])
                        nc.sync.dma_start(out=out_ext[ts(ct, 128), ts(hf, 512)], in_=out_sb)
                    return emit

                return comm, [once(p_unit(ct, hf)) for ct in range(4) for hf in range(2)]

            def do_attn(b, qkv, own_weave, fillers, fill_from_wci, carry=(),
                        defer_last=False, tail_weave=()):
                """Attention for batch b, software-pipelined.

                own_weave: weave this batch's own k/q/v units in by deadline
                (first-batch cold start). fillers: unconstrained unit closures
                (next batch's qkv) drained one per m-tile starting at chunk
                fill_from_wci. tail_weave: (mt, closure) pairs placed in the
                LAST chunk's loop (the hidden A2A + its projection).
                """
                loads, qT_sb, kT_sb, v_sb, pre, sched, v_units = qkv
                if not own_weave:
                    for u in pre + [u for _, u in sched] + v_units:
                        u()  # one-shot: no-op for units already woven elsewhere
                else:
                    for u in pre:
                        u()
                    for mt in range(4):
                        v_units[mt]()

                xT_sb = xtpool.tile([D, N], BF16, name="xT_sb", tag="xT_sb", bufs=1)
                with nc.named_scope(f"attn{b}"):
                    def scores(wci, mt):
                        # h0 on PE row-tile (0,0), h1 on (64,0): concurrent.
                        pss = psS.tile([128, WCH], F32, name="pss", tag="pss")
                        for h in range(2):
                            nc.tensor.matmul(
                                pss[:, ts(h, 512)],
                                kT_sb[ds(h * D, D), ts(mt, 128)],
                                qT_sb[ds(h * D, D), ds(wci * WCH + h * 512, 512)],
                                start=True, stop=True,
                            )
                        return pss

                    def make_norm(o_sb, r_sb, wci):
                        # deferred normalize: xT = o_sb * (1/r_sb); woven into
                        # the NEXT chunk's loop. The denominator row is
                        # replicated across D partitions on the (idle) GPSIMD
                        # engine so the PE never touches the normalize.
                        rbs = []

                        def stage_a():
                            for h in range(2):
                                rb = small.tile([D, 512], F32, name="rb", bufs=4)
                                nc.gpsimd.partition_broadcast(rb, r_sb[h], channels=D)
                                rbs.append(rb)

                        def stage_b():
                            for h in range(2):
                                rbi = small.tile([D, 512], F32, name="rbi", bufs=4)
                                nc.vector.reciprocal_approx_fast(rbi, rbs[h])
                                nc.vector.tensor_mul(
                                    xT_sb[:, ds(wci * WCH + h * 512, 512)],
                                    o_sb[h], rbi,
                                )
                            if b == 1:
                                # scatter this chunk's shard right away: the
                                # hidden A2A #1 takes slices 0-2, the tail
                                # A2A #2 takes only the last chunk's slice.
                                nc.sync.dma_start(
                                    out=a2a_in[0 if wci < 3 else 1][4 + wci],
                                    in_=xT_sb[:, ds(wci * WCH, WCH)],
                                )
                        return [stage_a, stage_b]

                    # software-pipelined: scores(mt+1) is emitted before PV(mt)
                    # so the in-order PE stream never stalls behind the exp.
                    pss_cur = scores(0, 0)
                    pending_norm = None
                    for wci in range(N // WCH):
                        pso = [
                            psO.tile([D + 1, 512], F32, name=f"pso{h}", tag=f"pso{h}",
                                     bufs=2 if h == 0 else 1)
                            for h in range(2)
                        ]
                        pv0 = None  # deferred first PV of the chunk
                        for mt in range(32):
                            if wci == N // WCH - 1:
                                for wmt, u in tail_weave:
                                    if wmt == mt:
                                        u()
                            if pending_norm is not None and mt in (2, 4):
                                pending_norm[(mt - 2) // 2]()
                            if carry and wci == 0 and mt in (6, 8, 10, 12):
                                ci = (mt - 6) // 2
                                if ci < len(carry):
                                    carry[ci]()
                            if own_weave and wci == 0:
                                # cold start: feed k/q/v units just ahead of
                                # use; all own units are done by end of wci 0
                                # so fillers (from wci 1) can't pool-deadlock.
                                for wmt, u in sched:
                                    if wmt == mt:
                                        u()
                                if mt + 4 < 32:
                                    v_units[mt + 4]()
                            if wci >= fill_from_wci and fillers:
                                fillers.pop(0)()
                            p_sb = ppool.tile([128, WCH], BF16, name="p_sb")
                            nc.scalar.activation(
                                p_sb[:, 0:XACT], pss_cur[:, 0:XACT], EXP_FUNC, scale=SCALE
                            )
                            if XDVE:
                                with nc.allow_low_precision(reason="schraudolph exp"):
                                    nc.vector.tensor_scalar(
                                        p_sb.bitcast(I16)[:, XACT:WCH],
                                        pss_cur[:, XACT:WCH],
                                        SCHRA_A, SCHRA_B,
                                        op0=mybir.AluOpType.mult,
                                        op1=mybir.AluOpType.add,
                                    )
                            if mt < 31:
                                pss_cur = scores(wci, mt + 1)
                            elif wci < N // WCH - 1:
                                pss_cur = scores(wci + 1, 0)
                            if mt == 0:
                                # defer PV(0) one iteration: the PE keeps two
                                # scores queued while the previous chunk's pso
                                # slots drain through their SBUF copies.
                                pv0 = p_sb
                                continue
                            if mt == 1:
                                for h in range(2):
                                    nc.tensor.matmul(
                                        pso[h], v_sb[:, 0, :], pv0[:, ts(h, 512)],
                                        start=True, stop=False,
                                    )
                            for h in range(2):
                                nc.tensor.matmul(
                                    pso[h], v_sb[:, mt, :], p_sb[:, ts(h, 512)],
                                    start=False, stop=(mt == 31),
                                )
                        # free the pso slots promptly via PSUM->SBUF copies;
                        # the rest of the normalize is deferred into the next
                        # chunk (or emitted now for the last chunk). The denom
                        # row is copied to a partition-0 tile because the
                        # custom-DVE reciprocal can't take partition offsets.
                        o_sb, r_sb = [], []
                        for h in range(2):
                            o = small.tile([D, 512], F32, name=f"o_sb{h}", tag=f"o_sb{h}")
                            nc.vector.tensor_copy(o, pso[h][0:D, :])
                            r = small.tile([1, 512], F32, name=f"r_sb{h}", tag=f"r_sb{h}")
                            nc.vector.tensor_copy(r, pso[h][D : D + 1, :])
                            o_sb.append(o)
                            r_sb.append(r)
                        pending_norm = make_norm(o_sb, r_sb, wci)
                        if wci == N // WCH - 1 and not defer_last:
                            pending_norm[0]()
                            pending_norm[1]()
                            pending_norm = None

                @once
                def scatter():
                    if b == 1:
                        return  # batch-1 shards scatter per-chunk in stage_b
                    for j in range(4):
                        nc.sync.dma_start(
                            out=a2a_in[0][j],
                            in_=xT_sb[:, ds(j * NSLICE, NSLICE)],
                        )

                if defer_last:
                    # hand the last chunk's normalize + this batch's scatter
                    # to the caller to weave into the next batch's attention,
                    # keeping the batch transition stall-free.
                    return [once(pending_norm[0]), once(pending_norm[1]), scatter]
                scatter()
                return []

            # Batch 0 weaves its own qkv units in by deadline (cold start);
            # batch 1's loads + qkv units fill batch 0's ACT-bound attention.
            # The combined A2A + this core's single-slice projection are the
            # only tail after batch 1's attention.
            qkv0 = qkv_units(0, paired=False)
            qkv1 = qkv_units(1, paired=True, defer_loads=True)
            fill0 = [qkv1[0]] + qkv1[4] + [u for _, u in qkv1[5]] + qkv1[6]
            carry0 = do_attn(0, qkv0, own_weave=True, fillers=fill0,
                             fill_from_wci=1, defer_last=True)
            for u in fill0:
                u()
            # A2A #1's trigger fires inside attn1's last chunk (its inputs
            # complete with chunk 2's deferred normalize at mt 4), so its
            # setup/barrier/transfer run on the CC engine under the last
            # chunk's compute without ever blocking the PE. The projection
            # off it (valid on cores 0-6) runs right after attn1, filling
            # A2A #2's barrier wait with real PE work — A2A #1's data always
            # lands before A2A #2's (same straggler, one chunk earlier).
            # Core 7 (whose slice IS the last chunk) projects off A2A #2.
            comm1, pu1 = proj_units(0, out_a)
            do_attn(1, qkv1, own_weave=False, fillers=[],
                    fill_from_wci=2, carry=carry0, tail_weave=[(6, comm1)])
            for u in carry0:
                u()
            comm2, pu2 = proj_units(1, out_b)
            comm2()
            for u in pu1:
                u()
            for u in pu2:
                u()

    nc.finalize()
    return nc


_NC_CACHE = {}


def _get_nc():
    if "nc" not in _NC_CACHE:
        _NC_CACHE["nc"] = build_nc()
    return _NC_CACHE["nc"]


def _chunk_major(xT):
    # [C, N] -> [8, 128, 4, 512]: (j, p, t, n) = xT[t*128+p, j*512+n]
    return np.ascontiguousarray(
        xT.reshape(4, 128, NCH, 512).transpose(2, 1, 0, 3)
    )


def _w_tiled(wT, d):
    # [C, d] -> [128, 4, d]: (p, t, :) = wT[t*128+p, :]
    return np.ascontiguousarray(wT.reshape(4, 128, d).transpose(1, 0, 2))


def _make_in_maps(query, key, value, Wq, Wk, Wv, Wp, bp):
    bf = ml_dtypes.bfloat16
    shared = {}
    for b in range(B):
        shared[f"queryT{b}"] = _chunk_major(query[b].T.astype(bf))
        shared[f"keyT{b}"] = _chunk_major(key[b].T.astype(bf))
        shared[f"valueT{b}"] = _chunk_major(value[b].T.astype(bf))
    shared["wpT"] = _w_tiled(Wp.T.astype(bf), C)
    shared["bp"] = np.ascontiguousarray(
        bp.astype(np.float32).reshape(4, 128).T
    )
    shared["vones"] = np.ones((128, 32), bf)

    in_maps = []
    for j in range(NCORES):
        m = dict(shared)
        m["wqT"] = _w_tiled(Wq[j * D : (j + 1) * D, :].T.astype(bf), D)
        m["wkT"] = _w_tiled(Wk[j * D : (j + 1) * D, :].T.astype(bf), D)
        m["wvT"] = _w_tiled(Wv[j * D : (j + 1) * D, :].T.astype(bf), D)
        in_maps.append(m)
    return in_maps


def run(inputs, trace=False):
    inputs = {k: np.asarray(v) for k, v in inputs.items()}
    nc = _get_nc()
    in_maps = _make_in_maps(**inputs)
    res = run_bass_kernel_spmd(nc, in_maps, core_ids=list(range(NCORES)), trace=trace)
    full = np.empty((B, N, C), np.float32)
    for s in range(4):
        full[0, s * NSLICE : (s + 1) * NSLICE, :] = res.results[s]["out_a"].T
    for s in range(3):
        full[1, s * NSLICE : (s + 1) * NSLICE, :] = res.results[4 + s]["out_a"].T
    full[1, 3 * NSLICE :, :] = res.results[7]["out_b"].T
    return full, res


def kernel(**inputs):
    return run(inputs, trace=False)[0]



# revision 11
# speedup vs baseline: 1.2435x; 1.0163x over previous
"""Distributed attention kernel for 8 TRN2 NeuronCores.

Problem: B=2, N=M=4096, C=512, H=8 heads, D=64.
  q/k/v = linear(query/key/value); attn = softmax(q k^T / sqrt(D)); out = proj(attn v) + bias

Sharding: 1 head per core (tensor parallel over heads); each core runs both
batches for its head. A single 8-core AllToAll at the end swaps head-shards
for (batch, token-slice) shards: dest core j<4 gets batch-0 token-slice j,
core j>=4 gets batch-1 slice j-4; each core runs the output projection for
exactly one slice. The host gathers out from core j into full[b, slice].

Layouts (per core, head h = core index):
  inputs staged chunk-major + bf16 on host: [8, 128, 4, 512]
    (chunk j, partition p, t, n) = x.T[t*128+p, j*512+n] so each 512-column
    chunk load is one contiguous-per-partition DMA (cheap descriptors).
  qT, kT: (D=64, N) = WxT.T @ xT            (PE, bf16; q/k pairs run as
                                             concurrent column-tiles when the
                                             input chunks are resident)
  v:      (M, D) natural + a ones column     (so P@V also yields softmax denom)
  sT:     (m-tile 128, n-chunk 1024) = k q^T (PSUM f32, two 512 matmuls)
  P:      exp(sT / sqrt(D))                  (one wide ACT op; no max-sub:
                                              scores ~ N(0,1), max ~6)
  oT:     (65, 512) += v_aug.T @ P-half      (row 64 = sum_m P = softmax denom)
  xT:     oT[0:64] * (1/denom)               (denom bcast via GPSIMD
                                              partition_broadcast, recip on DVE)
  A2A:    shard j = xT_b(j)[:, slice]        (bf16, 1MB/rank, one collective)
  proj:   out(c', n) = WpT.T @ xfull + bp    (bias per-partition on ACT)
"""

import numpy as np
import ml_dtypes

import concourse.bass as bass
import concourse.mybir as mybir
import concourse.tile as tile
from concourse import bacc
from concourse.bass import ds, ts
from concourse.bass_utils import run_bass_kernel_spmd

BF16 = mybir.dt.bfloat16
F32 = mybir.dt.float32

B, N, M, C, H, D = 2, 4096, 4096, 512, 8, 64
SCALE = D ** -0.5
NCORES = 8
NSLICE = N // 4          # 1024 tokens per core after A2A
WCH = 1024               # scores/exp chunk width
NCH = 8                  # 512-column input chunks per tensor
EXP_FUNC = mybir.ActivationFunctionType.Exp
ID_FUNC = mybir.ActivationFunctionType.Identity
I16 = mybir.dt.int16
# DVE takes the top XDVE columns of each 1024-wide exp via a bf16 Schraudolph:
# bf16bits(exp(s/8)) ~= round(s * (0.125*128*log2 e) + (127*128 + 0.75)).
# The sawtooth error (~3% on P) mostly cancels in the softmax ratio; the
# uniform-scale part cancels exactly.
XDVE = 448
XACT = WCH - XDVE
SCHRA_A = float(0.125 * 128 * np.log2(np.e))
SCHRA_B = float(127 * 128 + 0.75)


def build_nc():
    nc = bacc.Bacc(
        "TRN2", target_bir_lowering=False, debug=False, num_devices=NCORES
    )

    # Per-core DRAM parameters. Chunk-major activations are identical on every
    # core; weight slices are per-head.
    qryT = [nc.declare_dram_parameter(f"queryT{b}", [NCH, 128, 4, 512], BF16, isOutput=False) for b in range(B)]
    keyT = [nc.declare_dram_parameter(f"keyT{b}", [NCH, 128, 4, 512], BF16, isOutput=False) for b in range(B)]
    valT = [nc.declare_dram_parameter(f"valueT{b}", [NCH, 128, 4, 512], BF16, isOutput=False) for b in range(B)]
    wqT = nc.declare_dram_parameter("wqT", [128, 4, D], BF16, isOutput=False)
    wkT = nc.declare_dram_parameter("wkT", [128, 4, D], BF16, isOutput=False)
    wvT = nc.declare_dram_parameter("wvT", [128, 4, D], BF16, isOutput=False)
    wpT = nc.declare_dram_parameter("wpT", [128, 4, C], BF16, isOutput=False)
    bp = nc.declare_dram_parameter("bp", [128, 4], F32, isOutput=False)
    vones = nc.declare_dram_parameter("vones", [128, 32], BF16, isOutput=False)
    # Every core projects an equal share of every A2A phase: batch-0 tokens
    # split 8x512, batch-1 tokens 0-3071 split 8x384, batch-1 tail split 8x128.
    out_a = nc.declare_dram_parameter("out_a", [C, 512], F32, isOutput=True)
    out_b = nc.declare_dram_parameter("out_b", [C, 384], F32, isOutput=True)
    out_c = nc.declare_dram_parameter("out_c", [C, 128], F32, isOutput=True)

    with tile.TileContext(nc) as tc:
        with (
            tc.tile_pool(name="consts", bufs=1) as consts,
            tc.tile_pool(name="inputs", bufs=1) as inputs,
            tc.tile_pool(name="qk", bufs=2) as qkpool,
            tc.tile_pool(name="vp", bufs=2) as vpool,
            tc.tile_pool(name="ppool", bufs=3) as ppool,
            tc.tile_pool(name="small", bufs=2) as small,
            tc.tile_pool(name="xt", bufs=1) as xtpool,
            tc.tile_pool(name="psS", bufs=2, space="PSUM") as psS,   # scores, 2 banks/slot
            tc.tile_pool(name="psO", bufs=1, space="PSUM") as psO,   # attention accumulators
            tc.tile_pool(name="psB", bufs=1, space="PSUM") as psB,   # qkv/proj matmuls
            tc.tile_pool(name="dram", bufs=1, space="DRAM") as dram,
        ):
            # ---- constants / weights (host pre-tiled: plain contiguous loads)
            wq_sb = consts.tile([128, 4, D], BF16, name="wq_sb")
            wk_sb = consts.tile([128, 4, D], BF16, name="wk_sb")
            wv_sb = consts.tile([128, 4, D], BF16, name="wv_sb")
            wp_sb = consts.tile([128, 4, C], BF16, name="wp_sb")
            bp_sb = consts.tile([128, 4], F32, name="bp_sb")
            nc.sync.dma_start(out=wq_sb, in_=wqT[:])
            nc.sync.dma_start(out=wk_sb, in_=wkT[:])
            nc.sync.dma_start(out=wv_sb, in_=wvT[:])
            nc.sync.dma_start(out=wp_sb, in_=wpT[:])
            nc.sync.dma_start(out=bp_sb, in_=bp[:])

            # Three token-split A2As, each spreading its tokens evenly over all
            # 8 cores so every projection phase runs 8-way parallel:
            #   #0: batch-0 all 4096 tokens (8x512)  — fires in attn1 chunk 0,
            #       transfer + warmup hide under attn1's body.
            #   #1: batch-1 tokens 0-3071   (8x384)  — fires at attn1 chunk 3
            #       mt 6 (after chunk 2's deferred norm), hides under chunk 3.
            #   #2: batch-1 tokens 3072-4095 (8x128) — the only exposed tail:
            #       128KB transfer + a 4-matmul projection per core.
            A2A_W = [512, 384, 128]
            a2a_in = [dram.tile([NCORES, D, w], BF16, name=f"a2a_in{x}") for x, w in enumerate(A2A_W)]
            a2a_out = [dram.tile([NCORES, D, w], BF16, name=f"a2a_out{x}") for x, w in enumerate(A2A_W)]

            def once(fn):
                done = [False]
                def wrapper():
                    if not done[0]:
                        done[0] = True
                        fn()
                return wrapper

            def qkv_units(b, paired, defer_loads=False):
                """Allocate tiles; return (loads, unit closures).

                Each unit closure emits one PSUM-sized piece of the q/k/v
                projections. The caller weaves them into the attention loop so
                the in-order PE stream never sits on a long block of
                projection matmuls while ACT is idle.

                paired: emit q+k of the same chunk as two concurrent PE
                column-tiles (outputs to partition halves of one PSUM bank) —
                ~2x faster, but couples their input-chunk deadlines, so it is
                only used when the input data is already resident (batch 1)
                or guaranteed-early (first two chunks of batch 0).
                """
                qry_sb = inputs.tile([128, NCH, 4, 512], BF16, name="qry_sb")
                key_sb = inputs.tile([128, NCH, 4, 512], BF16, name="key_sb")
                val_sb = inputs.tile([128, NCH, 4, 512], BF16, name="val_sb")
                # qT/kT live duplicated in BOTH partition halves (rows 0:64 and
                # 64:128) so the two 512-wide score halves of each m-tile can
                # run as concurrent PE ROW-tiles (tile_position (0,0)/(64,0)):
                # K=64 only fills half the array, so the pair costs ~one unit.
                qT_sb = qkpool.tile([128, N], BF16, name="qT_sb", tag="qT_sb")
                kT_sb = qkpool.tile([128, M], BF16, name="kT_sb", tag="kT_sb")
                v_sb = vpool.tile([128, 32, D + 1], BF16, name="v_sb")

                @once
                def loads():
                    # chunk loads are contiguous per partition (1 descriptor
                    # row): issue cost ~100ns each instead of 0.7-6us.
                    # Order: the pre-loop units (kq0, kq1, v0-3) first, then
                    # key/val interleaved (their use-deadlines advance with
                    # the m-tile loop), q chunks 2+ last (needed a full
                    # scores-chunk later).
                    with nc.named_scope(f"qkv{b}"):
                        def chunk(dst, src, j):
                            nc.sync.dma_start(out=dst[:, j], in_=src[j])
                        chunk(key_sb, keyT[b], 0)
                        chunk(qry_sb, qryT[b], 0)
                        chunk(key_sb, keyT[b], 1)
                        chunk(qry_sb, qryT[b], 1)
                        chunk(val_sb, valT[b], 0)
                        chunk(val_sb, valT[b], 1)
                        nc.sync.dma_start(out=v_sb[:, :, D], in_=vones[:])
                        for j in range(2, NCH):
                            chunk(key_sb, keyT[b], j)
                            chunk(val_sb, valT[b], j)
                        for j in range(2, NCH):
                            chunk(qry_sb, qryT[b], j)
                if not defer_loads:
                    loads()

                def pair_unit(s1, s2):
                    # two 64-row projection pieces as concurrent column-tiles:
                    # s1 -> output partitions 0:64 (PE col groups 0-1),
                    # s2 -> 64:128 (groups 2-3); disjoint sub-arrays run
                    # simultaneously, so the pair costs ~one unit.
                    (d1, w1, s1_, i1), (d2, w2, s2_, i2) = s1, s2
                    def emit():
                        ps = psB.tile([128, 512], F32, name="pskq", tag="psB")
                        for t in range(4):
                            nc.tensor.matmul(
                                ps[0:D, :], w1[:, t, :], s1_[:, i1, t, :],
                                start=(t == 0), stop=(t == 3),
                            )
                            nc.tensor.matmul(
                                ps[D:128, :], w2[:, t, :], s2_[:, i2, t, :],
                                start=(t == 0), stop=(t == 3),
                            )
                        with nc.allow_low_precision(reason="bf16 scores"):
                            nc.vector.tensor_copy(d1[0:D, ds(i1 * 512, 512)], ps[0:D, :])
                            nc.vector.tensor_copy(d1[D:128, ds(i1 * 512, 512)], ps[0:D, :])
                            nc.vector.tensor_copy(d2[0:D, ds(i2 * 512, 512)], ps[D:128, :])
                            nc.vector.tensor_copy(d2[D:128, ds(i2 * 512, 512)], ps[D:128, :])
                    return emit

                def K(i):
                    return (kT_sb, wk_sb, key_sb, i)

                def Q(i):
                    return (qT_sb, wq_sb, qry_sb, i)

                def v_unit(mt):
                    def emit():
                        psv = psB.tile([128, D], F32, name="psv", tag="psB")
                        for t in range(4):
                            nc.tensor.matmul(
                                psv, val_sb[:, mt // 4, t, ds((mt % 4) * 128, 128)],
                                wv_sb[:, t, :],
                                start=(t == 0), stop=(t == 3),
                            )
                        nc.vector.tensor_copy(v_sb[:, mt, 0:D], psv)
                    return emit

                if paired:
                    # batch-1 fillers: all inputs resident, pair freely
                    pre = [once(pair_unit(K(i), Q(i))) for i in range(NCH)]
                    sched = []
                else:
                    # batch 0 (cold start): chunks 0/1 arrive first and pair
                    # k+q; later chunks pair k+k (key data lands early) and
                    # q+q (both deadlines a full scores-chunk away), so every
                    # unit's inputs beat its weave slot.
                    pre = [once(pair_unit(K(0), Q(0))), once(pair_unit(K(1), Q(1)))]
                    # weave each pair at the latest slot that still beats its
                    # consumer (k chunks i feed scores m-tiles 4i): later
                    # slots let ready scores/PV work run ahead of the input
                    # DMA instead of stalling the PE queue on it.
                    sched = [
                        (4, once(pair_unit(K(2), K(3)))),
                        (10, once(pair_unit(K(4), K(5)))),
                        (16, once(pair_unit(K(6), K(7)))),
                        (20, once(pair_unit(Q(2), Q(3)))),
                        (24, once(pair_unit(Q(4), Q(5)))),
                        (28, once(pair_unit(Q(6), Q(7)))),
                    ]
                v_units = [once(v_unit(mt)) for mt in range(32)]
                return loads, qT_sb, kT_sb, v_sb, pre, sched, v_units

            def proj_units(x, out_ext):
                """A2A #x + xf-load closure, plus this core's projection."""
                w = A2A_W[x]
                xf = [
                    xtpool.tile([128, w], BF16, name=f"xf{x}{t}", tag=f"xf{x}{t}")
                    for t in range(4)
                ]

                @once
                def comm():
                    with nc.named_scope(f"proj{x}"):
                        nc.gpsimd.collective_compute(
                            "AllToAll",
                            mybir.AluOpType.bypass,
                            replica_groups=[list(range(NCORES))],
                            ins=[a2a_in[x].opt()],
                            outs=[a2a_out[x].opt()],
                        )
                        for t in range(4):
                            nc.sync.dma_start(
                                out=xf[t],
                                in_=a2a_out[x][2 * t : 2 * t + 2].rearrange("s d n -> (s d) n"),
                            )

                def p_unit(ct):
                    def emit():
                        psp = psB.tile([128, 512], F32, name="psp", tag="psB")
                        for et in range(4):
                            nc.tensor.matmul(
                                psp[:, 0:w], wp_sb[:, et, ts(ct, 128)], xf[et],
                                start=(et == 0), stop=(et == 3),
                            )
                        out_sb = small.tile([128, 512], F32, name="out_sb", bufs=4)
                        nc.scalar.activation(out_sb[:, 0:w], psp[:, 0:w], ID_FUNC, bias=bp_sb[:, ct : ct + 1])
                        nc.sync.dma_start(out=out_ext[ts(ct, 128), :], in_=out_sb[:, 0:w])
                    return emit

                return comm, [once(p_unit(ct)) for ct in range(4)]

            def do_attn(b, qkv, own_weave, fillers, fill_from_wci, carry=(),
                        defer_last=False, tail_weave=()):
                """Attention for batch b, software-pipelined.

                own_weave: weave this batch's own k/q/v units in by deadline
                (first-batch cold start). fillers: unconstrained unit closures
                (next batch's qkv) drained one per m-tile starting at chunk
                fill_from_wci. tail_weave: (mt, closure) pairs placed in the
                LAST chunk's loop (the hidden A2A + its projection).
                """
                loads, qT_sb, kT_sb, v_sb, pre, sched, v_units = qkv
                if not own_weave:
                    for u in pre + [u for _, u in sched] + v_units:
                        u()  # one-shot: no-op for units already woven elsewhere
                else:
                    for u in pre:
                        u()
                    for mt in range(4):
                        v_units[mt]()

                xT_sb = xtpool.tile([D, N], BF16, name="xT_sb", tag="xT_sb", bufs=1)
                with nc.named_scope(f"attn{b}"):
                    def scores(wci, mt):
                        # h0 on PE row-tile (0,0), h1 on (64,0): concurrent.
                        pss = psS.tile([128, WCH], F32, name="pss", tag="pss")
                        for h in range(2):
                            nc.tensor.matmul(
                                pss[:, ts(h, 512)],
                                kT_sb[ds(h * D, D), ts(mt, 128)],
                                qT_sb[ds(h * D, D), ds(wci * WCH + h * 512, 512)],
                                start=True, stop=True,
                            )
                        return pss

                    def make_norm(o_sb, r_sb, wci):
                        # deferred normalize: xT = o_sb * (1/r_sb); woven into
                        # the NEXT chunk's loop. The denominator row is
                        # replicated across D partitions on the (idle) GPSIMD
                        # engine so the PE never touches the normalize.
                        rbs = []

                        def stage_a():
                            for h in range(2):
                                rb = small.tile([D, 512], F32, name="rb", bufs=4)
                                nc.gpsimd.partition_broadcast(rb, r_sb[h], channels=D)
                                rbs.append(rb)

                        def stage_b():
                            for h in range(2):
                                rbi = small.tile([D, 512], F32, name="rbi", bufs=4)
                                nc.vector.reciprocal_approx_fast(rbi, rbs[h])
                                nc.vector.tensor_mul(
                                    xT_sb[:, ds(wci * WCH + h * 512, 512)],
                                    o_sb[h], rbi,
                                )
                            if b == 1:
                                # scatter this chunk right away: chunks 0-2
                                # feed the hidden A2A #1 (8x384 token split,
                                # chunk spans 2-3 shards), chunk 3 feeds the
                                # tail A2A #2 (8x128).
                                t0 = wci * WCH
                                if wci < 3:
                                    for j in range(t0 // 384, (t0 + WCH - 1) // 384 + 1):
                                        lo = max(t0, j * 384)
                                        hi = min(t0 + WCH, (j + 1) * 384)
                                        nc.sync.dma_start(
                                            out=a2a_in[1][j][:, ds(lo - j * 384, hi - lo)],
                                            in_=xT_sb[:, ds(lo, hi - lo)],
                                        )
                                else:
                                    for j in range(NCORES):
                                        nc.sync.dma_start(
                                            out=a2a_in[2][j],
                                            in_=xT_sb[:, ds(t0 + j * 128, 128)],
                                        )
                        return [stage_a, stage_b]

                    # software-pipelined: scores(mt+1) is emitted before PV(mt)
                    # so the in-order PE stream never stalls behind the exp.
                    pss_cur = scores(0, 0)
                    pending_norm = None
                    for wci in range(N // WCH):
                        pso = [
                            psO.tile([D + 1, 512], F32, name=f"pso{h}", tag=f"pso{h}",
                                     bufs=2 if h == 0 else 1)
                            for h in range(2)
                        ]
                        pv0 = None  # deferred first PV of the chunk
                        for mt in range(32):
                            if wci == N // WCH - 1:
                                for wmt, u in tail_weave:
                                    if wmt == mt:
                                        u()
                            if pending_norm is not None and mt in (2, 4):
                                pending_norm[(mt - 2) // 2]()
                            if carry and wci == 0 and mt in (6, 8, 10, 12):
                                ci = (mt - 6) // 2
                                if ci < len(carry):
                                    carry[ci]()
                            if own_weave and wci == 0:
                                # cold start: feed k/q/v units just ahead of
                                # use; all own units are done by end of wci 0
                                # so fillers (from wci 1) can't pool-deadlock.
                                for wmt, u in sched:
                                    if wmt == mt:
                                        u()
                                if mt + 4 < 32:
                                    v_units[mt + 4]()
                            if wci >= fill_from_wci and fillers:
                                fillers.pop(0)()
                            p_sb = ppool.tile([128, WCH], BF16, name="p_sb")
                            nc.scalar.activation(
                                p_sb[:, 0:XACT], pss_cur[:, 0:XACT], EXP_FUNC, scale=SCALE
                            )
                            if XDVE:
                                with nc.allow_low_precision(reason="schraudolph exp"):
                                    nc.vector.tensor_scalar(
                                        p_sb.bitcast(I16)[:, XACT:WCH],
                                        pss_cur[:, XACT:WCH],
                                        SCHRA_A, SCHRA_B,
                                        op0=mybir.AluOpType.mult,
                                        op1=mybir.AluOpType.add,
                                    )
                            if mt < 31:
                                pss_cur = scores(wci, mt + 1)
                            elif wci < N // WCH - 1:
                                pss_cur = scores(wci + 1, 0)
                            if mt == 0:
                                # defer PV(0) one iteration: the PE keeps two
                                # scores queued while the previous chunk's pso
                                # slots drain through their SBUF copies.
                                pv0 = p_sb
                                continue
                            if mt == 1:
                                for h in range(2):
                                    nc.tensor.matmul(
                                        pso[h], v_sb[:, 0, :], pv0[:, ts(h, 512)],
                                        start=True, stop=False,
                                    )
                            for h in range(2):
                                nc.tensor.matmul(
                                    pso[h], v_sb[:, mt, :], p_sb[:, ts(h, 512)],
                                    start=False, stop=(mt == 31),
                                )
                        # free the pso slots promptly via PSUM->SBUF copies;
                        # the rest of the normalize is deferred into the next
                        # chunk (or emitted now for the last chunk). The denom
                        # row is copied to a partition-0 tile because the
                        # custom-DVE reciprocal can't take partition offsets.
                        o_sb, r_sb = [], []
                        for h in range(2):
                            o = small.tile([D, 512], F32, name=f"o_sb{h}", tag=f"o_sb{h}")
                            nc.vector.tensor_copy(o, pso[h][0:D, :])
                            r = small.tile([1, 512], F32, name=f"r_sb{h}", tag=f"r_sb{h}")
                            nc.vector.tensor_copy(r, pso[h][D : D + 1, :])
                            o_sb.append(o)
                            r_sb.append(r)
                        pending_norm = make_norm(o_sb, r_sb, wci)
                        if wci == N // WCH - 1 and not defer_last:
                            pending_norm[0]()
                            pending_norm[1]()
                            pending_norm = None

                @once
                def scatter():
                    if b == 1:
                        return  # batch-1 shards scatter per-chunk in stage_b
                    for j in range(NCORES):
                        nc.sync.dma_start(
                            out=a2a_in[0][j],
                            in_=xT_sb[:, ds(j * 512, 512)],
                        )

                if defer_last:
                    # hand the last chunk's normalize + this batch's scatter
                    # to the caller to weave into the next batch's attention,
                    # keeping the batch transition stall-free.
                    return [once(pending_norm[0]), once(pending_norm[1]), scatter]
                scatter()
                return []

            # Batch 0 weaves its own qkv units in by deadline (cold start);
            # batch 1's loads + qkv units fill batch 0's ACT-bound attention.
            # The combined A2A + this core's single-slice projection are the
            # only tail after batch 1's attention.
            qkv0 = qkv_units(0, paired=False)
            qkv1 = qkv_units(1, paired=True, defer_loads=True)
            fill0 = [qkv1[0]] + qkv1[4] + [u for _, u in qkv1[5]] + qkv1[6]
            carry0 = do_attn(0, qkv0, own_weave=True, fillers=fill0,
                             fill_from_wci=1, defer_last=True)
            for u in fill0:
                u()
            # A2A #0 (batch-0 tokens) triggers in attn1 chunk 0 right after
            # batch-0's carried norm + scatter (carry slot mt 12); its warmup
            # + transfer hide under attn1 chunks 0-2 and its projection units
            # weave into chunk 3 (after 8 no-op pad slots so the xf loads have
            # landed). A2A #1 (batch-1 tokens 0-3071) triggers at chunk 3 mt 6
            # and hides under the rest of chunk 3. Only A2A #2 (128 tokens per
            # core) plus two short projection bursts remain after attn1.
            comm1, pu1 = proj_units(0, out_a)
            comm2, pu2 = proj_units(1, out_b)
            comm3, pu3 = proj_units(2, out_c)
            noop = lambda: None
            do_attn(1, qkv1, own_weave=False, fillers=[noop] * 8 + pu1,
                    fill_from_wci=3, carry=carry0 + [comm1],
                    tail_weave=[(6, comm2)])
            for u in carry0:
                u()
            comm1()
            comm2()
            comm3()
            for u in pu1:
                u()
            for u in pu2:
                u()
            for u in pu3:
                u()

    nc.finalize()
    return nc


_NC_CACHE = {}


def _get_nc():
    if "nc" not in _NC_CACHE:
        _NC_CACHE["nc"] = build_nc()
    return _NC_CACHE["nc"]


def _chunk_major(xT):
    # [C, N] -> [8, 128, 4, 512]: (j, p, t, n) = xT[t*128+p, j*512+n]
    return np.ascontiguousarray(
        xT.reshape(4, 128, NCH, 512).transpose(2, 1, 0, 3)
    )


def _w_tiled(wT, d):
    # [C, d] -> [128, 4, d]: (p, t, :) = wT[t*128+p, :]
    return np.ascontiguousarray(wT.reshape(4, 128, d).transpose(1, 0, 2))


def _make_in_maps(query, key, value, Wq, Wk, Wv, Wp, bp):
    bf = ml_dtypes.bfloat16
    shared = {}
    for b in range(B):
        shared[f"queryT{b}"] = _chunk_major(query[b].T.astype(bf))
        shared[f"keyT{b}"] = _chunk_major(key[b].T.astype(bf))
        shared[f"valueT{b}"] = _chunk_major(value[b].T.astype(bf))
    shared["wpT"] = _w_tiled(Wp.T.astype(bf), C)
    shared["bp"] = np.ascontiguousarray(
        bp.astype(np.float32).reshape(4, 128).T
    )
    shared["vones"] = np.ones((128, 32), bf)

    in_maps = []
    for j in range(NCORES):
        m = dict(shared)
        m["wqT"] = _w_tiled(Wq[j * D : (j + 1) * D, :].T.astype(bf), D)
        m["wkT"] = _w_tiled(Wk[j * D : (j + 1) * D, :].T.astype(bf), D)
        m["wvT"] = _w_tiled(Wv[j * D : (j + 1) * D, :].T.astype(bf), D)
        in_maps.append(m)
    return in_maps


def run(inputs, trace=False):
    inputs = {k: np.asarray(v) for k, v in inputs.items()}
    nc = _get_nc()
    in_maps = _make_in_maps(**inputs)
    res = run_bass_kernel_spmd(nc, in_maps, core_ids=list(range(NCORES)), trace=trace)
    full = np.empty((B, N, C), np.float32)
    for j in range(NCORES):
        full[0, j * 512 : (j + 1) * 512, :] = res.results[j]["out_a"].T
        full[1, j * 384 : (j + 1) * 384, :] = res.results[j]["out_b"].T
        full[1, 3072 + j * 128 : 3072 + (j + 1) * 128, :] = res.results[j]["out_c"].T
    return full, res


def kernel(**inputs):
    return run(inputs, trace=False)[0]



# revision 16
# speedup vs baseline: 1.3322x; 1.0714x over previous
"""Distributed attention kernel for 8 TRN2 NeuronCores.

Problem: B=2, N=M=4096, C=512, H=8 heads, D=64.
  q/k/v = linear(query/key/value); attn = softmax(q k^T / sqrt(D)); out = proj(attn v) + bias

Sharding: 1 head per core (tensor parallel over heads); each core runs both
batches for its head. A single 8-core AllToAll at the end swaps head-shards
for (batch, token-slice) shards: dest core j<4 gets batch-0 token-slice j,
core j>=4 gets batch-1 slice j-4; each core runs the output projection for
exactly one slice. The host gathers out from core j into full[b, slice].

Layouts (per core, head h = core index):
  inputs staged chunk-major + bf16 on host: [8, 128, 4, 512]
    (chunk j, partition p, t, n) = x.T[t*128+p, j*512+n] so each 512-column
    chunk load is one contiguous-per-partition DMA (cheap descriptors).
  qT, kT: (D=64, N) = WxT.T @ xT            (PE, bf16; q/k pairs run as
                                             concurrent column-tiles when the
                                             input chunks are resident)
  v:      (M, D) natural + a ones column     (so P@V also yields softmax denom)
  sT:     (m-tile 128, n-chunk 1024) = k q^T (PSUM f32, two 512 matmuls)
  P:      exp(sT / sqrt(D))                  (one wide ACT op; no max-sub:
                                              scores ~ N(0,1), max ~6)
  oT:     (65, 512) += v_aug.T @ P-half      (row 64 = sum_m P = softmax denom)
  xT:     oT[0:64] * (1/denom)               (denom bcast via GPSIMD
                                              partition_broadcast, recip on DVE)
  A2A:    shard j = xT_b(j)[:, slice]        (bf16, 1MB/rank, one collective)
  proj:   out(c', n) = WpT.T @ xfull + bp    (bias per-partition on ACT)
"""

import numpy as np
import ml_dtypes

import concourse.bass as bass
import concourse.mybir as mybir
import concourse.tile as tile
from concourse import bacc
from concourse.bass import ds, ts
from concourse.bass_utils import run_bass_kernel_spmd

BF16 = mybir.dt.bfloat16
F32 = mybir.dt.float32

B, N, M, C, H, D = 2, 4096, 4096, 512, 8, 64
SCALE = D ** -0.5
NCORES = 8
NSLICE = N // 4          # 1024 tokens per core after A2A
WCH = 1024               # scores/exp chunk width
NCH = 8                  # 512-column input chunks per tensor
EXP_FUNC = mybir.ActivationFunctionType.Exp
ID_FUNC = mybir.ActivationFunctionType.Identity
I16 = mybir.dt.int16
# DVE takes the top XDVE columns of each 1024-wide exp via a bf16 Schraudolph:
# bf16bits(exp(s/8)) ~= round(s * (0.125*128*log2 e) + (127*128 + 0.75)).
# The sawtooth error (~3% on P) mostly cancels in the softmax ratio; the
# uniform-scale part cancels exactly.
XDVE = 448
XACT = WCH - XDVE
SCHRA_A = float(0.125 * 128 * np.log2(np.e))
SCHRA_B = float(127 * 128 + 0.75)


def build_nc():
    nc = bacc.Bacc(
        "TRN2", target_bir_lowering=False, debug=False, num_devices=NCORES
    )

    # Per-core DRAM parameters. Chunk-major activations are identical on every
    # core; weight slices are per-head.
    qryT = [nc.declare_dram_parameter(f"queryT{b}", [NCH, 128, 4, 512], BF16, isOutput=False) for b in range(B)]
    keyT = [nc.declare_dram_parameter(f"keyT{b}", [NCH, 128, 4, 512], BF16, isOutput=False) for b in range(B)]
    valT = [nc.declare_dram_parameter(f"valueT{b}", [NCH, 128, 4, 512], BF16, isOutput=False) for b in range(B)]
    wqT = nc.declare_dram_parameter("wqT", [128, 4, D], BF16, isOutput=False)
    wkT = nc.declare_dram_parameter("wkT", [128, 4, D], BF16, isOutput=False)
    wvT = nc.declare_dram_parameter("wvT", [128, 4, D], BF16, isOutput=False)
    wpT = nc.declare_dram_parameter("wpT", [128, 4, C], BF16, isOutput=False)
    bp = nc.declare_dram_parameter("bp", [128, 4], F32, isOutput=False)
    vones = nc.declare_dram_parameter("vones", [128, 32], BF16, isOutput=False)
    # Every core projects an equal share of every A2A phase: batch-0 tokens
    # split 8x512, batch-1 tokens 0-3071 split 8x384, batch-1 tail split 8x128.
    out_a = nc.declare_dram_parameter("out_a", [C, 512], F32, isOutput=True)
    out_b = nc.declare_dram_parameter("out_b", [C, 384], F32, isOutput=True)
    out_c = nc.declare_dram_parameter("out_c", [C, 128], F32, isOutput=True)

    with tile.TileContext(nc) as tc:
        with (
            tc.tile_pool(name="consts", bufs=1) as consts,
            tc.tile_pool(name="inputs", bufs=1) as inputs,
            tc.tile_pool(name="qk", bufs=2) as qkpool,
            tc.tile_pool(name="vp", bufs=2) as vpool,
            tc.tile_pool(name="ppool", bufs=3) as ppool,
            tc.tile_pool(name="small", bufs=2) as small,
            tc.tile_pool(name="xt", bufs=1) as xtpool,
            tc.tile_pool(name="psS", bufs=2, space="PSUM") as psS,   # scores, 2 banks/slot
            tc.tile_pool(name="psO", bufs=1, space="PSUM") as psO,   # attention accumulators
            tc.tile_pool(name="psB", bufs=1, space="PSUM") as psB,   # qkv/proj matmuls
            tc.tile_pool(name="dram", bufs=1, space="DRAM") as dram,
        ):
            # ---- constants / weights (host pre-tiled: plain contiguous loads)
            wq_sb = consts.tile([128, 4, D], BF16, name="wq_sb")
            wk_sb = consts.tile([128, 4, D], BF16, name="wk_sb")
            wv_sb = consts.tile([128, 4, D], BF16, name="wv_sb")
            wp_sb = consts.tile([128, 4, C], BF16, name="wp_sb")
            bp_sb = consts.tile([128, 4], F32, name="bp_sb")
            nc.sync.dma_start(out=wq_sb, in_=wqT[:])
            nc.sync.dma_start(out=wk_sb, in_=wkT[:])
            nc.sync.dma_start(out=wv_sb, in_=wvT[:])
            nc.sync.dma_start(out=wp_sb, in_=wpT[:])
            nc.sync.dma_start(out=bp_sb, in_=bp[:])

            # Three token-split A2As, each spreading its tokens evenly over all
            # 8 cores so every projection phase runs 8-way parallel:
            #   #0: batch-0 all 4096 tokens (8x512)  — fires in attn1 chunk 0,
            #       transfer + warmup hide under attn1's body.
            #   #1: batch-1 tokens 0-3071   (8x384)  — fires at attn1 chunk 3
            #       mt 6 (after chunk 2's deferred norm), hides under chunk 3.
            #   #2: batch-1 tokens 3072-4095 (8x128) — the only exposed tail:
            #       128KB transfer + a 4-matmul projection per core.
            A2A_W = [512, 384, 128]
            a2a_in = [dram.tile([NCORES, D, w], BF16, name=f"a2a_in{x}") for x, w in enumerate(A2A_W)]
            a2a_out = [dram.tile([NCORES, D, w], BF16, name=f"a2a_out{x}") for x, w in enumerate(A2A_W)]

            def once(fn):
                done = [False]
                def wrapper():
                    if not done[0]:
                        done[0] = True
                        fn()
                return wrapper

            def qkv_units(b, paired, defer_loads=False):
                """Allocate tiles; return (loads, unit closures).

                Each unit closure emits one PSUM-sized piece of the q/k/v
                projections. The caller weaves them into the attention loop so
                the in-order PE stream never sits on a long block of
                projection matmuls while ACT is idle.

                paired: emit q+k of the same chunk as two concurrent PE
                column-tiles (outputs to partition halves of one PSUM bank) —
                ~2x faster, but couples their input-chunk deadlines, so it is
                only used when the input data is already resident (batch 1)
                or guaranteed-early (first two chunks of batch 0).
                """
                qry_sb = inputs.tile([128, NCH, 4, 512], BF16, name="qry_sb")
                key_sb = inputs.tile([128, NCH, 4, 512], BF16, name="key_sb")
                val_sb = inputs.tile([128, NCH, 4, 512], BF16, name="val_sb")
                # qT/kT live duplicated in BOTH partition halves (rows 0:64 and
                # 64:128) so the two 512-wide score halves of each m-tile can
                # run as concurrent PE ROW-tiles (tile_position (0,0)/(64,0)):
                # K=64 only fills half the array, so the pair costs ~one unit.
                qT_sb = qkpool.tile([128, N], BF16, name="qT_sb", tag="qT_sb")
                kT_sb = qkpool.tile([128, M], BF16, name="kT_sb", tag="kT_sb")
                v_sb = vpool.tile([128, 32, D + 1], BF16, name="v_sb")

                @once
                def loads():
                    # chunk loads are contiguous per partition (1 descriptor
                    # row): issue cost ~100ns each instead of 0.7-6us.
                    # Order: the pre-loop units (kq0, kq1, v0-3) first, then
                    # key/val interleaved (their use-deadlines advance with
                    # the m-tile loop), q chunks 2+ last (needed a full
                    # scores-chunk later).
                    with nc.named_scope(f"qkv{b}"):
                        def chunk(dst, src, j):
                            nc.sync.dma_start(out=dst[:, j], in_=src[j])
                        chunk(key_sb, keyT[b], 0)
                        chunk(qry_sb, qryT[b], 0)
                        chunk(key_sb, keyT[b], 1)
                        chunk(qry_sb, qryT[b], 1)
                        chunk(val_sb, valT[b], 0)
                        chunk(val_sb, valT[b], 1)
                        nc.sync.dma_start(out=v_sb[:, :, D], in_=vones[:])
                        for j in range(2, NCH):
                            chunk(key_sb, keyT[b], j)
                            chunk(val_sb, valT[b], j)
                        for j in range(2, NCH):
                            chunk(qry_sb, qryT[b], j)
                if not defer_loads:
                    loads()

                def pair_unit(s1, s2):
                    # two 64-row projection pieces as concurrent column-tiles:
                    # s1 -> output partitions 0:64 (PE col groups 0-1),
                    # s2 -> 64:128 (groups 2-3); disjoint sub-arrays run
                    # simultaneously, so the pair costs ~one unit.
                    (d1, w1, s1_, i1), (d2, w2, s2_, i2) = s1, s2
                    def emit():
                        ps = psB.tile([128, 512], F32, name="pskq", tag="psB")
                        for t in range(4):
                            nc.tensor.matmul(
                                ps[0:D, :], w1[:, t, :], s1_[:, i1, t, :],
                                start=(t == 0), stop=(t == 3),
                            )
                            nc.tensor.matmul(
                                ps[D:128, :], w2[:, t, :], s2_[:, i2, t, :],
                                start=(t == 0), stop=(t == 3),
                            )
                        with nc.allow_low_precision(reason="bf16 scores"):
                            nc.vector.tensor_copy(d1[0:D, ds(i1 * 512, 512)], ps[0:D, :])
                            nc.vector.tensor_copy(d1[D:128, ds(i1 * 512, 512)], ps[0:D, :])
                            nc.vector.tensor_copy(d2[0:D, ds(i2 * 512, 512)], ps[D:128, :])
                            nc.vector.tensor_copy(d2[D:128, ds(i2 * 512, 512)], ps[D:128, :])
                    return emit

                def K(i):
                    return (kT_sb, wk_sb, key_sb, i)

                def Q(i):
                    return (qT_sb, wq_sb, qry_sb, i)

                def v_unit(mt):
                    def emit():
                        psv = psB.tile([128, D], F32, name="psv", tag="psB")
                        for t in range(4):
                            nc.tensor.matmul(
                                psv, val_sb[:, mt // 4, t, ds((mt % 4) * 128, 128)],
                                wv_sb[:, t, :],
                                start=(t == 0), stop=(t == 3),
                            )
                        nc.vector.tensor_copy(v_sb[:, mt, 0:D], psv)
                    return emit

                if paired:
                    # batch-1 fillers: all inputs resident, pair freely
                    pre = [once(pair_unit(K(i), Q(i))) for i in range(NCH)]
                    sched = []
                else:
                    # batch 0 (cold start): chunks 0/1 arrive first and pair
                    # k+q; later chunks pair k+k (key data lands early) and
                    # q+q (both deadlines a full scores-chunk away), so every
                    # unit's inputs beat its weave slot.
                    pre = [once(pair_unit(K(0), Q(0))), once(pair_unit(K(1), Q(1)))]
                    # weave each pair at the latest slot that still beats its
                    # consumer (k chunks i feed scores m-tiles 4i): later
                    # slots let ready scores/PV work run ahead of the input
                    # DMA instead of stalling the PE queue on it.
                    sched = [
                        (4, once(pair_unit(K(2), K(3)))),
                        (10, once(pair_unit(K(4), K(5)))),
                        (16, once(pair_unit(K(6), K(7)))),
                        (20, once(pair_unit(Q(2), Q(3)))),
                        (24, once(pair_unit(Q(4), Q(5)))),
                        (28, once(pair_unit(Q(6), Q(7)))),
                    ]
                v_units = [once(v_unit(mt)) for mt in range(32)]
                return loads, qT_sb, kT_sb, v_sb, pre, sched, v_units

            def proj_units(x, out_ext):
                """A2A #x + xf-load closure, plus this core's projection."""
                w = A2A_W[x]
                xf = [
                    xtpool.tile([128, w], BF16, name=f"xf{x}{t}", tag=f"xf{x}{t}")
                    for t in range(4)
                ]

                @once
                def comm():
                    with nc.named_scope(f"proj{x}"):
                        nc.gpsimd.collective_compute(
                            "AllToAll",
                            mybir.AluOpType.bypass,
                            replica_groups=[list(range(NCORES))],
                            ins=[a2a_in[x].opt()],
                            outs=[a2a_out[x].opt()],
                        )

                # xf loads are issued SEPARATELY, once the transfer is known
                # to be done: a dma_start that waits on the collective parks
                # in the in-order sync DMA queue and blocks every later DMA
                # (scatters, outputs) behind the transfer.
                @once
                def loads():
                    with nc.named_scope(f"projL{x}"):
                        for t in range(4):
                            nc.sync.dma_start(
                                out=xf[t],
                                in_=a2a_out[x][2 * t : 2 * t + 2].rearrange("s d n -> (s d) n"),
                            )

                def p_unit(ct):
                    def emit():
                        psp = psB.tile([128, 512], F32, name="psp", tag="psB")
                        for et in range(4):
                            nc.tensor.matmul(
                                psp[:, 0:w], wp_sb[:, et, ts(ct, 128)], xf[et],
                                start=(et == 0), stop=(et == 3),
                            )
                        out_sb = small.tile([128, 512], F32, name="out_sb", bufs=4)
                        nc.scalar.activation(out_sb[:, 0:w], psp[:, 0:w], ID_FUNC, bias=bp_sb[:, ct : ct + 1])
                        nc.sync.dma_start(out=out_ext[ts(ct, 128), :], in_=out_sb[:, 0:w])
                    return emit

                return comm, loads, [once(p_unit(ct)) for ct in range(4)]

            def do_attn(b, qkv, own_weave, fillers, fill_from_wci, carry=(),
                        defer_last=False, tail_weave=()):
                """Attention for batch b, software-pipelined.

                own_weave: weave this batch's own k/q/v units in by deadline
                (first-batch cold start). fillers: unconstrained unit closures
                (next batch's qkv) drained one per m-tile starting at chunk
                fill_from_wci. tail_weave: (mt, closure) pairs placed in the
                LAST chunk's loop (the hidden A2A + its projection).
                """
                loads, qT_sb, kT_sb, v_sb, pre, sched, v_units = qkv
                if not own_weave:
                    for u in pre + [u for _, u in sched] + v_units:
                        u()  # one-shot: no-op for units already woven elsewhere
                else:
                    for u in pre:
                        u()
                    for mt in range(4):
                        v_units[mt]()

                xT_sb = xtpool.tile([D, N], BF16, name="xT_sb", tag="xT_sb", bufs=1)
                with nc.named_scope(f"attn{b}"):
                    def scores(wci, mt):
                        # h0 on PE row-tile (0,0), h1 on (64,0): concurrent.
                        pss = psS.tile([128, WCH], F32, name="pss", tag="pss")
                        for h in range(2):
                            nc.tensor.matmul(
                                pss[:, ts(h, 512)],
                                kT_sb[ds(h * D, D), ts(mt, 128)],
                                qT_sb[ds(h * D, D), ds(wci * WCH + h * 512, 512)],
                                start=True, stop=True,
                            )
                        return pss

                    def make_norm(o_sb, r_sb, wci):
                        # deferred normalize: xT = o_sb * (1/r_sb); woven into
                        # the NEXT chunk's loop. The denominator row is
                        # replicated across D partitions on the (idle) GPSIMD
                        # engine so the PE never touches the normalize.
                        rbs = []

                        def stage_a():
                            for h in range(2):
                                rb = small.tile([D, 512], F32, name="rb", bufs=4)
                                nc.gpsimd.partition_broadcast(rb, r_sb[h], channels=D)
                                rbs.append(rb)

                        def stage_b():
                            for h in range(2):
                                rbi = small.tile([D, 512], F32, name="rbi", bufs=4)
                                nc.vector.reciprocal_approx_fast(rbi, rbs[h])
                                nc.vector.tensor_mul(
                                    xT_sb[:, ds(wci * WCH + h * 512, 512)],
                                    o_sb[h], rbi,
                                )
                            if b == 1:
                                # scatter this chunk right away: chunks 0-2
                                # feed the hidden A2A #1 (8x384 token split,
                                # chunk spans 2-3 shards), chunk 3 feeds the
                                # tail A2A #2 (8x128).
                                t0 = wci * WCH
                                if wci < 3:
                                    for j in range(t0 // 384, (t0 + WCH - 1) // 384 + 1):
                                        lo = max(t0, j * 384)
                                        hi = min(t0 + WCH, (j + 1) * 384)
                                        nc.sync.dma_start(
                                            out=a2a_in[1][j][:, ds(lo - j * 384, hi - lo)],
                                            in_=xT_sb[:, ds(lo, hi - lo)],
                                        )
                                else:
                                    # one strided DMA: shard j gets tokens
                                    # [t0+128j, t0+128j+128) of every head dim
                                    nc.sync.dma_start(
                                        out=a2a_in[2].rearrange("j d t -> d j t"),
                                        in_=xT_sb[:, ds(t0, 1024)].rearrange(
                                            "d (j t) -> d j t", j=NCORES
                                        ),
                                    )
                        return [stage_a, stage_b]

                    # software-pipelined: scores(mt+1) is emitted before PV(mt)
                    # so the in-order PE stream never stalls behind the exp.
                    pss_cur = scores(0, 0)
                    pending_norm = None
                    for wci in range(N // WCH):
                        pso = [
                            psO.tile([D + 1, 512], F32, name=f"pso{h}", tag=f"pso{h}",
                                     bufs=2 if h == 0 else 1)
                            for h in range(2)
                        ]
                        pv0 = None  # deferred first PV of the chunk
                        for mt in range(32):
                            if wci == N // WCH - 1:
                                for wmt, u in tail_weave:
                                    if wmt == mt:
                                        u()
                            if pending_norm is not None and mt in (2, 4):
                                pending_norm[(mt - 2) // 2]()
                            if carry and wci == 0 and mt in (6, 8, 10, 12):
                                ci = (mt - 6) // 2
                                if ci < len(carry):
                                    carry[ci]()
                            if own_weave and wci == 0:
                                # cold start: feed k/q/v units just ahead of
                                # use; all own units are done by end of wci 0
                                # so fillers (from wci 1) can't pool-deadlock.
                                for wmt, u in sched:
                                    if wmt == mt:
                                        u()
                                if mt + 4 < 32:
                                    v_units[mt + 4]()
                            if wci >= fill_from_wci and fillers:
                                fillers.pop(0)()
                            p_sb = ppool.tile([128, WCH], BF16, name="p_sb")
                            nc.scalar.activation(
                                p_sb[:, 0:XACT], pss_cur[:, 0:XACT], EXP_FUNC, scale=SCALE
                            )
                            if XDVE:
                                with nc.allow_low_precision(reason="schraudolph exp"):
                                    nc.vector.tensor_scalar(
                                        p_sb.bitcast(I16)[:, XACT:WCH],
                                        pss_cur[:, XACT:WCH],
                                        SCHRA_A, SCHRA_B,
                                        op0=mybir.AluOpType.mult,
                                        op1=mybir.AluOpType.add,
                                    )
                            if mt < 31:
                                pss_cur = scores(wci, mt + 1)
                            elif wci < N // WCH - 1:
                                pss_cur = scores(wci + 1, 0)
                            if mt == 0:
                                # defer PV(0) one iteration: the PE keeps two
                                # scores queued while the previous chunk's pso
                                # slots drain through their SBUF copies.
                                pv0 = p_sb
                                continue
                            if mt == 1:
                                for h in range(2):
                                    nc.tensor.matmul(
                                        pso[h], v_sb[:, 0, :], pv0[:, ts(h, 512)],
                                        start=True, stop=False,
                                    )
                            for h in range(2):
                                nc.tensor.matmul(
                                    pso[h], v_sb[:, mt, :], p_sb[:, ts(h, 512)],
                                    start=False, stop=(mt == 31),
                                )
                        # free the pso slots promptly via PSUM->SBUF copies;
                        # the rest of the normalize is deferred into the next
                        # chunk (or emitted now for the last chunk). The denom
                        # row is copied to a partition-0 tile because the
                        # custom-DVE reciprocal can't take partition offsets.
                        o_sb, r_sb = [], []
                        for h in range(2):
                            o = small.tile([D, 512], F32, name=f"o_sb{h}", tag=f"o_sb{h}")
                            nc.vector.tensor_copy(o, pso[h][0:D, :])
                            r = small.tile([1, 512], F32, name=f"r_sb{h}", tag=f"r_sb{h}")
                            nc.vector.tensor_copy(r, pso[h][D : D + 1, :])
                            o_sb.append(o)
                            r_sb.append(r)
                        pending_norm = make_norm(o_sb, r_sb, wci)
                        if wci == N // WCH - 1 and not defer_last:
                            pending_norm[0]()
                            pending_norm[1]()
                            pending_norm = None

                @once
                def scatter():
                    if b == 1:
                        return  # batch-1 shards scatter per-chunk in stage_b
                    nc.sync.dma_start(
                        out=a2a_in[0].rearrange("j d t -> d j t"),
                        in_=xT_sb.rearrange("d (j t) -> d j t", j=NCORES),
                    )

                if defer_last:
                    # hand the last chunk's normalize + this batch's scatter
                    # to the caller to weave into the next batch's attention,
                    # keeping the batch transition stall-free.
                    return [once(pending_norm[0]), once(pending_norm[1]), scatter]
                scatter()
                return []

            # Batch 0 weaves its own qkv units in by deadline (cold start);
            # batch 1's loads + qkv units fill batch 0's ACT-bound attention.
            # The combined A2A + this core's single-slice projection are the
            # only tail after batch 1's attention.
            qkv0 = qkv_units(0, paired=False)
            qkv1 = qkv_units(1, paired=True, defer_loads=True)
            fill0 = [qkv1[0]] + qkv1[4] + [u for _, u in qkv1[5]] + qkv1[6]
            carry0 = do_attn(0, qkv0, own_weave=True, fillers=fill0,
                             fill_from_wci=1, defer_last=True)
            for u in fill0:
                u()
            # A2A #0 (batch-0 tokens) triggers in attn1 chunk 0 right after
            # batch-0's carried norm + scatter (carry slot mt 12); its warmup
            # + transfer hide under attn1 chunks 0-2 and its projection units
            # weave into chunk 3 (after 8 no-op pad slots so the xf loads have
            # landed). A2A #1 (batch-1 tokens 0-3071) triggers at chunk 3 mt 6
            # and hides under the rest of chunk 3. Only A2A #2 (128 tokens per
            # core) plus two short projection bursts remain after attn1.
            comm1, load1, pu1 = proj_units(0, out_a)
            comm2, load2, pu2 = proj_units(1, out_b)
            comm3, load3, pu3 = proj_units(2, out_c)
            noop = lambda: None
            do_attn(1, qkv1, own_weave=False,
                    fillers=[noop] * 7 + [load1] + pu1,
                    fill_from_wci=3, carry=carry0 + [comm1],
                    tail_weave=[(6, comm2), (20, load2)])
            for u in carry0:
                u()
            comm1()
            load1()
            comm2()
            load2()
            comm3()
            for u in pu1:
                u()
            for u in pu2:
                u()
            load3()
            for u in pu3:
                u()

    nc.finalize()
    return nc


_NC_CACHE = {}


def _get_nc():
    if "nc" not in _NC_CACHE:
        _NC_CACHE["nc"] = build_nc()
    return _NC_CACHE["nc"]


def _chunk_major(xT):
    # [C, N] -> [8, 128, 4, 512]: (j, p, t, n) = xT[t*128+p, j*512+n]
    return np.ascontiguousarray(
        xT.reshape(4, 128, NCH, 512).transpose(2, 1, 0, 3)
    )


def _w_tiled(wT, d):
    # [C, d] -> [128, 4, d]: (p, t, :) = wT[t*128+p, :]
    return np.ascontiguousarray(wT.reshape(4, 128, d).transpose(1, 0, 2))


def _make_in_maps(query, key, value, Wq, Wk, Wv, Wp, bp):
    bf = ml_dtypes.bfloat16
    shared = {}
    for b in range(B):
        shared[f"queryT{b}"] = _chunk_major(query[b].T.astype(bf))
        shared[f"keyT{b}"] = _chunk_major(key[b].T.astype(bf))
        shared[f"valueT{b}"] = _chunk_major(value[b].T.astype(bf))
    shared["wpT"] = _w_tiled(Wp.T.astype(bf), C)
    shared["bp"] = np.ascontiguousarray(
        bp.astype(np.float32).reshape(4, 128).T
    )
    shared["vones"] = np.ones((128, 32), bf)

    in_maps = []
    for j in range(NCORES):
        m = dict(shared)
        m["wqT"] = _w_tiled(Wq[j * D : (j + 1) * D, :].T.astype(bf), D)
        m["wkT"] = _w_tiled(Wk[j * D : (j + 1) * D, :].T.astype(bf), D)
        m["wvT"] = _w_tiled(Wv[j * D : (j + 1) * D, :].T.astype(bf), D)
        in_maps.append(m)
    return in_maps


def run(inputs, trace=False):
    inputs = {k: np.asarray(v) for k, v in inputs.items()}
    nc = _get_nc()
    in_maps = _make_in_maps(**inputs)
    res = run_bass_kernel_spmd(nc, in_maps, core_ids=list(range(NCORES)), trace=trace)
    full = np.empty((B, N, C), np.float32)
    for j in range(NCORES):
        full[0, j * 512 : (j + 1) * 512, :] = res.results[j]["out_a"].T
        full[1, j * 384 : (j + 1) * 384, :] = res.results[j]["out_b"].T
        full[1, 3072 + j * 128 : 3072 + (j + 1) * 128, :] = res.results[j]["out_c"].T
    return full, res


def kernel(**inputs):
    return run(inputs, trace=False)[0]

